# revision 12
# baseline (speedup 1.0000x reference)
"""BGFusionBlock Trainium2 kernel (Bass/Tile, 8 NeuronCores, SPMD).

Shapes: aligned_feat [4, 8, 64, 128, 128] f32, w1/w2 [64, 64, 3, 3],
b1/b2 [64], wf [64, 512, 1, 1], bf [64].  Output [4, 64, 128, 128] f32.

Math:
  emb     = conv3x3(x, w2)   per frame           (biases: b2 enters via esum)
  emb_ref = conv3x3(x, w1)   per frame           (b1 cancels in the softmax)
  scores[b,t,p] = <emb_ref[b,t,:,p], sum_j emb[b,j,:,p] + 8*b2>
  attn = softmax(2*scores, axis=t)
  out  = leaky_relu(conv1x1(aligned_feat * attn, wf) + bf, 0.1)

Distribution: shard H across the 8 cores (16 rows each, 1-row halo baked
into the per-core input by the host).  All compute is core-local.

Per-core plan (each (b, half-slab of 8 rows) is one unit, px = 8*128 = 1024):
 - x staged as [128p, 8t, 9r, 130c] f32r tiles: partitions 0:64 = channel c
   at slab row r, partitions 64:128 = channel c at slab row r+1.  K=128
   matmuls cover vertical tap pairs (0,dj)+(1,dj); M=128 packs BOTH convs
   (w1|w2, swapped on odd frames).
 - The di=2 tail is packed via a second fp16 tensor xt [128p, 8t, 8r, 130c]:
   partitions 0:64 = row r+2 (col c-1), partitions 64:128 = row r+2 (col c),
   so ONE K=128 matmul covers taps (2,0)+(2,1) and one K=64 matmul (hi bank)
   covers (2,2).  5 matmuls per 512-px chunk per frame (vs 6 unpacked).
 - cps PSUM tiles span 2 banks so each frame needs only 2 scalar-engine
   evictions of [64, 1024] (ref half / emb half).
 - emb summed over t by identity-matmuls into PSUM; scores by 2x-scaled
   ones-matmuls over channel products; score transpose via PE; softmax on
   DVE; attn transposed back via the DMA XBAR (fp16); attn broadcast to
   (t,c) partitions by 0/1-matmuls; 1x1 conv as a K=512 matmul in 4
   K-tiles; LeakyReLU on the vector engine.
 - Tail drain of the last unit keeps the PE HAM-warm with filler matmuls.
"""

import sys
import os
import numpy as np

if "/opt/trn_rl_repo" not in sys.path:
    sys.path.insert(0, "/opt/trn_rl_repo")

B, T, C, H, W = 4, 8, 64, 128, 128
NCORES = 8
RPC = H // NCORES          # rows per core (16)
HS = RPC // 2              # rows per half-slab (8)
PX = HS * W                # pixels per half-slab (1024)
NCH = PX // 512            # 512-wide chunks per half-slab (2)

_CACHE: dict = {}

# attn transpose back to [t, px]: True = fp16 DMA XBAR, False = PE transposes
USE_DMA_T = False


# ----------------------------------------------------------------------------
# host-side input staging
# ----------------------------------------------------------------------------

def _stage_inputs(aligned_feat):
    """[B,T,C,H,W] -> per-core main tiles [B,2,128,T,9,130] f32 and
    fp16 tail tiles [B,2,128,T,8,130]."""
    af = np.ascontiguousarray(aligned_feat, dtype=np.float32)
    P = np.zeros((B, T, C, H + 2, W), np.float32)
    P[:, :, :, 1:-1, :] = af
    # rows[k, s, r] = 16k + 8s + r  (padded row index of slab row r)
    rows = (16 * np.arange(NCORES)[:, None, None]
            + HS * np.arange(2)[None, :, None]
            + np.arange(9)[None, None, :])
    A = np.zeros((NCORES, B, 2, 128, T, 9, 130), np.float32)
    # lo bank: channel c at slab row r; hi bank: channel c at slab row r+1
    Vlo = P[:, :, :, rows, :]          # [B,T,C,8,2,9,W]
    Vhi = P[:, :, :, rows + 1, :]
    A[:, :, :, :64, :, :, 1:129] = Vlo.transpose(3, 0, 4, 2, 1, 5, 6)
    A[:, :, :, 64:, :, :, 1:129] = Vhi.transpose(3, 0, 4, 2, 1, 5, 6)

    # tail tensor: both banks hold row r+2; lo at col c-1, hi at col c
    rows2 = rows[:, :, :8] + 2         # [k, s, r=0..7]
    V2 = P[:, :, :, rows2, :]          # [B,T,C,8,2,8,W]
    V2 = V2.transpose(3, 0, 4, 2, 1, 5, 6)   # [k,B,2,C,T,8,W]
    XT = np.zeros((NCORES, B, 2, 128, T, 8, 130), np.float16)
    XT[:, :, :, :64, :, :, 1:129] = V2
    XT[:, :, :, 64:, :, :, 0:128] = V2
    return A, XT


def _make_consts(w1, b1, w2, b2, wf, bf):
    w1 = np.asarray(w1, np.float32); w2 = np.asarray(w2, np.float32)
    b2 = np.asarray(b2, np.float32)
    wf = np.asarray(wf, np.float32).reshape(C, T * C)
    bf = np.asarray(bf, np.float32)

    # conv lhsT: k = di*64 + cc (di in {0,1}); m = conv*64 + oc
    # (conv order swapped on odd frames so pair tiles assemble lane-locked)
    wta = np.zeros((128, 2, 3, 128), np.float32)
    wpair = np.zeros((128, 2, 128), np.float16)
    wlast = np.zeros((128, 2, 128), np.float16)
    for sw, (wa, wb) in enumerate([(w1, w2), (w2, w1)]):
        for dj in range(3):
            for di in range(2):
                wta[di * 64:(di + 1) * 64, sw, dj, :64] = wa[:, :, di, dj].T
                wta[di * 64:(di + 1) * 64, sw, dj, 64:] = wb[:, :, di, dj].T
        # xt lo bank = tap (2,0); hi bank = tap (2,1)
        wpair[:64, sw, :64] = wa[:, :, 2, 0].T
        wpair[:64, sw, 64:] = wb[:, :, 2, 0].T
        wpair[64:, sw, :64] = wa[:, :, 2, 1].T
        wpair[64:, sw, 64:] = wb[:, :, 2, 1].T
        # tap (2,2) from the hi bank (col c maps to w+1)
        wlast[64:, sw, :64] = wa[:, :, 2, 2].T
        wlast[64:, sw, 64:] = wb[:, :, 2, 2].T

    wsum = np.zeros((128, 128), np.float32)
    kk = np.arange(128)
    wsum[kk, kk % 64] = 1.0
    wsum[kk, kk % 64 + 64] = 1.0

    # score reduce; 2.0 = 1/TEMPERATURE folded in
    wsc = np.zeros((128, 4, 8), np.float32)
    for j in range(4):
        wsc[:64, j, 2 * j] = 2.0
        wsc[64:, j, 2 * j + 1] = 2.0

    we = np.zeros((8, 4, 128), np.float16 if USE_DMA_T else np.float32)
    for j in range(4):
        we[2 * j, j, :64] = 1.0
        we[2 * j + 1, j, 64:] = 1.0

    wwf = np.zeros((128, 4, 128), np.float32)
    for j in range(4):
        for p in range(2):
            blk = wf[:, (2 * j + p) * 64:(2 * j + p + 1) * 64].T  # [cc, oc]
            wwf[p * 64:(p + 1) * 64, j, :64] = blk
            wwf[p * 64:(p + 1) * 64, j, 64:] = blk

    bias = np.zeros((128, 2), np.float32)
    bias[:64, 0] = 8.0 * b2; bias[64:, 0] = 8.0 * b2
    bias[:64, 1] = bf; bias[64:, 1] = bf

    cc = {
        "wta": wta, "wpair": wpair, "wlast": wlast, "wsum": wsum,
        "wsc": wsc, "we": we, "wwf": wwf, "bias": bias,
        "idt8": np.eye(8, dtype=np.float32),
    }
    if not USE_DMA_T:
        cc["idt128"] = np.eye(128, dtype=np.float32)
    return cc


CONST_SHAPES = {
    "wta": (128, 2, 3, 128), "wpair": (128, 2, 128), "wlast": (128, 2, 128),
    "wsum": (128, 128), "wsc": (128, 4, 8), "we": (8, 4, 128),
    "wwf": (128, 4, 128), "bias": (128, 2), "idt8": (8, 8),
}
if not USE_DMA_T:
    CONST_SHAPES["idt128"] = (128, 128)


# ----------------------------------------------------------------------------
# kernel program
# ----------------------------------------------------------------------------

def build_nc():
    import concourse.bass as bass
    import concourse.tile as tile
    import concourse.mybir as mybir
    from concourse import bacc
    from contextlib import ExitStack

    f32 = mybir.dt.float32
    f32r = mybir.dt.float32r
    fp16 = mybir.dt.float16
    ACT = mybir.ActivationFunctionType
    ALU = mybir.AluOpType
    AX = mybir.AxisListType

    nc = bacc.Bacc("TRN2", target_bir_lowering=False, debug=False)

    CDT = {"wta": f32r, "wpair": fp16, "wlast": fp16, "wsum": f32r,
           "wsc": f32r, "we": fp16 if USE_DMA_T else f32r, "wwf": f32r,
           "bias": f32, "idt8": f32, "idt128": f32}
    x = nc.dram_tensor("x", [B, 2, 128, T, 9, 130], f32r,
                       kind="ExternalInput").ap()
    xtd = nc.dram_tensor("xt", [B, 2, 128, T, 8, 130], fp16,
                         kind="ExternalInput").ap()
    cst = {n: nc.dram_tensor(n, list(s), CDT[n], kind="ExternalInput").ap()
           for n, s in CONST_SHAPES.items()}
    out = nc.dram_tensor("out", [B, 128, HS, W], f32, kind="ExternalOutput").ap()

    with tile.TileContext(nc) as tc, ExitStack() as ctx:
        cp = ctx.enter_context(tc.tile_pool(name="const", bufs=1))
        wup_f = cp.tile([128, 512], f32, tag="wup_f")
        nc.gpsimd.memset(wup_f[:], 1.0)
        consts = {}

        def load_consts(names):
            for n in names:
                s = CONST_SHAPES[n]
                t = cp.tile(list(s), CDT[n], tag=n, name=f"c_{n}")
                nc.sync.dma_start(t[:], cst[n][:])
                consts[n] = t

        xx_pool = ctx.enter_context(tc.tile_pool(name="xx", bufs=4))
        xt_pool = ctx.enter_context(tc.tile_pool(name="xtp", bufs=3))
        ref_pool = ctx.enter_context(tc.tile_pool(name="ref", bufs=4))
        emb_pool = ctx.enter_context(tc.tile_pool(name="emb", bufs=2))
        sml_pool = ctx.enter_context(tc.tile_pool(name="sml", bufs=2))
        prod_pool = ctx.enter_context(tc.tile_pool(name="prodp", bufs=4))
        out_pool = ctx.enter_context(tc.tile_pool(name="outb", bufs=2))
        # PSUM: ps1 = 2 tiles x 2 banks (conv cps, double-buffered frames);
        # pss = 1 tile x 2 banks (emb sums / scps, unit-cycled);
        # ps3 = 2 tiles x 1 bank (psT / abc rotation).
        ps1 = ctx.enter_context(tc.tile_pool(name="ps1", bufs=2, space="PSUM"))
        pss = ctx.enter_context(tc.tile_pool(name="pss", bufs=1, space="PSUM"))
        ps3 = ctx.enter_context(tc.tile_pool(name="ps3", bufs=2, space="PSUM"))

        state = {}

        def r(ap):
            return ap.bitcast(f32r)

        def load_xx(u, frame_split=False):
            b, s = divmod(u, 2)
            xxh = []
            xth = []
            for hf in range(2):
                xt_ = xx_pool.tile([128, 4, 9, 130], f32r, tag="xx",
                                   name=f"xx{u}_{hf}")
                tt = xt_pool.tile([128, 4, 8, 130], fp16, tag="xt",
                                  name=f"xt{u}_{hf}")
                if frame_split:
                    for f in range(4):
                        nc.sync.dma_start(xt_[:, f], x[b, s, :, 4 * hf + f])
                        nc.sync.dma_start(tt[:, f], xtd[b, s, :, 4 * hf + f])
                else:
                    nc.sync.dma_start(xt_[:], x[b, s, :, 4 * hf:4 * (hf + 1)])
                    nc.sync.dma_start(tt[:], xtd[b, s, :, 4 * hf:4 * (hf + 1)])
                xxh.append(xt_)
                xth.append(tt)
            return xxh, xth

        def emit_a(u, inject, pre=None):
            """Conv/scores phase for unit u; inject[t] () emitted after
            frame t's convs (PE-stream interleaving for unit u-1)."""
            b, s = divmod(u, 2)
            if pre is None:
                xxh, xth = load_xx(u)
            else:
                xxh, xth = pre

            def xx(t):
                return xxh[t // 4][:, t % 4]

            def xt(t):
                return xth[t // 4][:, t % 4]

            sums = pss.tile([128, NCH, 512], f32, tag="pss", name=f"sum{u}")
            refs = []
            embs = []
            for t in range(T):
                j, sw = divmod(t, 2)
                if sw == 0:
                    refj = ref_pool.tile([128, NCH, 512], f32, tag="embref",
                                         name=f"ref{u}_{j}")
                    embj = emb_pool.tile([128, NCH, 512], f32r, tag="emb",
                                         name=f"emb{u}_{j}")
                    refs.append(refj)
                    embs.append(embj)
                refj, embj = refs[j], embs[j]
                cps = ps1.tile([128, NCH, 512], f32, tag="cps",
                               name=f"cps{u}_{t}")
                for dj in range(3):
                    for ch in range(NCH):
                        nc.tensor.matmul(
                            cps[:, ch], consts['wta'][:, sw, dj, :],
                            xx(t)[:, 4 * ch:4 * ch + 4, dj:dj + 128],
                            start=(dj == 0), stop=False)
                for ch in range(NCH):
                    nc.tensor.matmul(
                        cps[:, ch], consts['wpair'][:, sw, :],
                        xt(t)[:, 4 * ch:4 * ch + 4, 0:128],
                        start=False, stop=False)
                for ch in range(NCH):
                    nc.tensor.matmul(
                        cps[:, ch], consts['wlast'][64:128, sw, :],
                        xt(t)[64:128, 4 * ch:4 * ch + 4, 1:129],
                        start=False, stop=True)
                if sw == 0:
                    nc.scalar.activation(refj[0:64], cps[0:64], ACT.Copy)
                    nc.scalar.activation(embj[64:128], cps[64:128], ACT.Copy)
                else:
                    nc.scalar.activation(embj[0:64], cps[0:64], ACT.Copy)
                    nc.scalar.activation(refj[64:128], cps[64:128], ACT.Copy)
                if sw == 1:
                    for ch in range(NCH):
                        nc.tensor.matmul(sums[:, ch], consts['wsum'][:],
                                         embj[:, ch, :],
                                         start=(j == 0), stop=(j == 3))
                fn = inject.get(t)
                if fn:
                    fn()

            esum = sml_pool.tile([128, NCH, 512], f32, tag="esum",
                                 name=f"esum{u}")
            nc.scalar.activation(esum[:], sums[:], ACT.Identity,
                                 bias=consts['bias'][:, 0:1])
            scps = pss.tile([8, NCH, 512], f32, tag="pss", name=f"scps{u}")
            prods = []
            for j in range(4):
                prodj = prod_pool.tile([128, NCH, 512], f32r, tag="prod",
                                       name=f"prod{u}_{j}")
                nc.vector.tensor_mul(prodj[:].opt(), refs[j][:].opt(),
                                     esum[:].opt())
                prods.append(prodj)
            for j in range(4):
                for ch in range(NCH):
                    nc.tensor.matmul(scps[:, ch], consts['wsc'][:, j, :],
                                     prods[j][:, ch, :],
                                     start=(j == 0), stop=(j == 3))
            state[u] = {"xxh": xxh, "scps": scps}

        def filler(i):
            """One HAM-warming matmul on a scratch PSUM bank."""
            fps = ps3.tile([128, 4, 128], f32, tag="ps3", name=f"fill{i}",
                           bufs=2)
            nc.tensor.matmul(fps[:].opt(), r(wup_f[:, 0:128]), r(wup_f[:]),
                             start=True, stop=True)

        def b_pieces(u, tail=False):
            """B-phase for unit u as injectable pieces keyed by slot."""
            b, s = divmod(u, 2)
            st = state[u]

            def xx(t):
                return st["xxh"][t // 4][:, t % 4]

            def p_trans():
                scs = sml_pool.tile([8, NCH, 512], f32, tag="scs",
                                    name=f"scs{u}")
                nc.scalar.activation(scs[:], st["scps"][:], ACT.Copy)
                psT = ps3.tile([128, HS, 8], f32, tag="ps3", name=f"psT{u}")
                for h in range(HS):
                    nc.tensor.transpose(
                        psT[:, h, :],
                        scs[:].opt()[:, 128 * h:128 * (h + 1)], consts['idt8'][:])
                st["psT"] = psT

            def p_soft():
                from concourse.bass import broadcast_tensor_aps
                psT = st["psT"]
                nm = sml_pool.tile([128, HS, 1], f32, tag="nm",
                                   name=f"nm{u}")
                nc.vector.tensor_reduce(nm[:, :, 0], psT[:], axis=AX.X,
                                        op=ALU.max)
                sm = sml_pool.tile([128, HS, 8], f32, tag="sm",
                                   name=f"sm{u}")
                a0, a1 = broadcast_tensor_aps(psT[:], nm[:])
                nc.vector.tensor_tensor(sm[:], a0, a1, op=ALU.subtract)
                et = sml_pool.tile([128, HS, 8], f32, tag="et",
                                   name=f"et{u}")
                nc.scalar.activation(et[:].opt(), sm[:].opt(), ACT.Exp)
                den = sml_pool.tile([128, HS, 1], f32, tag="den",
                                    name=f"den{u}")
                nc.vector.tensor_reduce(den[:, :, 0], et[:], axis=AX.X,
                                        op=ALU.add)
                rec = sml_pool.tile([128, HS, 1], f32, tag="rec",
                                    name=f"rec{u}")
                nc.vector.reciprocal(rec[:], den[:])
                if USE_DMA_T:
                    attnT = sml_pool.tile([128, HS, 16], fp16, tag="attnT",
                                          name=f"attnT{u}")
                    # cols 8:16 transpose to attn partitions 8:16 (never
                    # read); zero them so the XBAR DMA doesn't read
                    # uninitialized SBUF
                    nc.gpsimd.memset(attnT[:, :, 8:16], 0.0)
                    b0, b1_ = broadcast_tensor_aps(et[:], rec[:])
                    nc.vector.tensor_tensor(attnT[:, :, 0:8], b0, b1_,
                                            op=ALU.mult)
                else:
                    attnT = sml_pool.tile([128, HS, 8], f32, tag="attnT",
                                          name=f"attnT{u}")
                    b0, b1_ = broadcast_tensor_aps(et[:], rec[:])
                    nc.vector.tensor_tensor(attnT[:], b0, b1_, op=ALU.mult)
                st["attnT"] = attnT

            def p_btrans():
                if USE_DMA_T:
                    attn = sml_pool.tile([16, HS, 128], fp16, tag="attn",
                                         name=f"attn{u}")
                    nc.sync.dma_start(attn[:], st["attnT"][:], transpose=True)
                    st["attn"] = attn
                    return
                attnT = st["attnT"]
                attn_ps = [ps3.tile([8, 512], f32, tag="ps3",
                                    name=f"attnps{u}_{ch}")
                           for ch in range(NCH)]
                for h in range(HS):
                    nc.tensor.transpose(
                        attn_ps[h // 4][:, 128 * (h % 4):128 * (h % 4 + 1)],
                        attnT[:, h, :], consts['idt128'][:])
                attn = sml_pool.tile([8, HS, 128], f32r, tag="attn",
                                     name=f"attn{u}")
                for ch in range(NCH):
                    nc.scalar.activation(attn[0:8, 4 * ch:4 * (ch + 1), :],
                                         attn_ps[ch][:], ACT.Copy)
                st["attn"] = attn

            def p_wj(j):
                def fn():
                    attn = st["attn"]
                    wts = st.setdefault("wts", [])
                    for ch in range(NCH):
                        abc = ps3.tile([128, 4, 128], f32, tag="ps3",
                                       name=f"abc{u}_{j}_{ch}")
                        nc.tensor.matmul(abc[:].opt(), consts['we'][:, j, :],
                                         attn[0:8, 4 * ch:4 * (ch + 1), :].opt(),
                                         start=True, stop=True)
                        wt = sml_pool.tile([128, 4, 128], f32r, tag="wt",
                                           bufs=5, name=f"wt{u}_{j}_{ch}")
                        nc.vector.tensor_mul(
                            wt[0:64],
                            xx(2 * j)[0:64, 1 + 4 * ch:5 + 4 * ch, 1:129]
                            .bitcast(f32), abc[0:64])
                        nc.vector.tensor_mul(
                            wt[64:128],
                            xx(2 * j + 1)[64:128, 4 * ch:4 + 4 * ch, 1:129]
                            .bitcast(f32), abc[64:128])
                        wts.append(wt)
                return fn

            def p_wf():
                wts = st["wts"]
                ops = ps1.tile([128, NCH, 512], f32, tag="cps",
                               name=f"ops{u}")
                for j in range(4):
                    for ch in range(NCH):
                        nc.tensor.matmul(ops[:, ch], consts['wwf'][:, j, :],
                                         wts[2 * j + ch][:].opt(),
                                         start=(j == 0), stop=(j == 3))
                outbuf = state.get(("ob", b))
                if outbuf is None:
                    outbuf = out_pool.tile([128, HS, W], f32, tag="outbuf",
                                           name=f"outbuf{b}")
                    state[("ob", b)] = outbuf
                yb = sml_pool.tile([128, NCH, 512], f32, tag="yb",
                                   name=f"yb{u}")
                nc.scalar.activation(
                    yb[64 * s:64 * (s + 1)],
                    ops[64 * s:64 * (s + 1)],
                    ACT.Identity, bias=consts['bias'][64 * s:64 * (s + 1), 1:2])
                nc.vector.scalar_tensor_tensor(
                    outbuf[64 * s:64 * (s + 1)].opt(),
                    yb[64 * s:64 * (s + 1)].opt(), 0.1,
                    yb[64 * s:64 * (s + 1)].opt(),
                    op0=ALU.mult, op1=ALU.max)
                nc.sync.dma_start(out[b, 64 * s:64 * (s + 1)],
                                  outbuf[64 * s:64 * (s + 1)])
                del state[u]
                if s == 1:
                    del state[("ob", b)]

            return {0: p_trans, 1: p_soft, 2: p_btrans,
                    3: p_wj(0), 4: p_wj(1), 5: p_wj(2), 6: p_wj(3),
                    7: p_wf}

        # --- startup: weights first, then frame 0 of unit 0, warmup MMs ---
        load_consts(["wta"])
        b0_, s0_ = 0, 0
        xx0 = xx_pool.tile([128, 4, 9, 130], f32r, tag="xx", name="xx0_0")
        nc.sync.dma_start(xx0[:, 0], x[b0_, s0_, :, 0])
        load_consts(["wpair", "wlast"])
        xt0 = xt_pool.tile([128, 4, 8, 130], fp16, tag="xt", name="xt0_0")
        nc.sync.dma_start(xt0[:, 0], xtd[b0_, s0_, :, 0])
        for f in range(1, 4):
            nc.sync.dma_start(xx0[:, f], x[b0_, s0_, :, f])
            nc.sync.dma_start(xt0[:, f], xtd[b0_, s0_, :, f])
        xx1 = xx_pool.tile([128, 4, 9, 130], f32r, tag="xx", name="xx0_1")
        xt1 = xt_pool.tile([128, 4, 8, 130], fp16, tag="xt", name="xt0_1")
        for f in range(4):
            nc.sync.dma_start(xx1[:, f], x[b0_, s0_, :, 4 + f])
            nc.sync.dma_start(xt1[:, f], xtd[b0_, s0_, :, 4 + f])
        load_consts([n for n in CONST_SHAPES
                     if n not in ("wta", "wpair", "wlast")])

        wup = ps1.tile([128, NCH, 512], f32, tag="cps", name="wup")
        for i in range(16):
            nc.tensor.matmul(wup[:, 0], r(wup_f[:, 0:128]), r(wup_f[:]),
                             start=True, stop=True)

        emit_a(0, {}, ([xx0, xx1], [xt0, xt1]))
        for u in range(1, 8):
            emit_a(u, b_pieces(u - 1))
        # drain last unit; fillers keep the PE HAM-warm through the chain
        tail = b_pieces(7, tail=True)
        for k in range(8):
            tail[k]()
            if k < 7:
                nfill = 3 if k < 3 else 4
                for i in range(nfill):
                    filler(10 * k + i)

    nc.compile()
    return nc


# ----------------------------------------------------------------------------
# entry point
# ----------------------------------------------------------------------------

def kernel(aligned_feat, w1, b1, w2, b2, wf, bf):
    from concourse import bass_utils

    if "nc" not in _CACHE:
        _CACHE["nc"] = build_nc()
    nc = _CACHE["nc"]

    A, XT = _stage_inputs(aligned_feat)
    consts = _make_consts(w1, b1, w2, b2, wf, bf)
    in_maps = [{"x": A[k], "xt": XT[k], **consts} for k in range(NCORES)]

    res = bass_utils.run_bass_kernel_spmd(nc, in_maps, core_ids=list(range(NCORES)))
    outs = [res.results[k]["out"] for k in range(NCORES)]  # [B,128,HS,W]

    full = np.empty((B, C, H, W), np.float32)
    for k in range(NCORES):
        o = outs[k]
        for s in range(2):
            full[:, :, RPC * k + HS * s: RPC * k + HS * (s + 1), :] = \
                o[:, 64 * s:64 * (s + 1), :, :]
    return full


# revision 25
# speedup vs baseline: 231109.8260x; 231109.8260x over previous
"""BGFusionBlock Trainium2 kernel (Bass/Tile, 8 NeuronCores, SPMD).

Shapes: aligned_feat [4, 8, 64, 128, 128] f32, w1/w2 [64, 64, 3, 3],
b1/b2 [64], wf [64, 512, 1, 1], bf [64].  Output [4, 64, 128, 128] f32.

Math:
  emb     = conv3x3(x, w2)   per frame           (biases: b2 enters via esum)
  emb_ref = conv3x3(x, w1)   per frame           (b1 cancels in the softmax)
  scores[b,t,p] = <emb_ref[b,t,:,p], sum_j emb[b,j,:,p] + 8*b2>
  attn = softmax(2*scores, axis=t)
  out  = leaky_relu(conv1x1(aligned_feat * attn, wf) + bf, 0.1)

Distribution: shard H across the 8 cores (16 rows each, 1-row halo baked
into the per-core input by the host).  All compute is core-local.

Per-core plan (each (b, half-slab of 8 rows) is one unit, px = 8*128 = 1024):
 - x staged as [128p, 8t, 9r, 130c] f32r tiles: partitions 0:64 = channel c
   at slab row r, partitions 64:128 = channel c at slab row r+1.  K=128
   matmuls cover vertical tap pairs (0,dj)+(1,dj); M=128 packs BOTH convs
   (w1|w2, swapped on odd frames).
 - The di=2 tail is packed via a second fp16 tensor xt [128p, 8t, 8r, 130c]:
   partitions 0:64 = row r+2 (col c-1), partitions 64:128 = row r+2 (col c),
   so ONE K=128 matmul covers taps (2,0)+(2,1) and one K=64 matmul (hi bank)
   covers (2,2).  5 matmuls per 512-px chunk per frame (vs 6 unpacked).
 - cps PSUM tiles span 2 banks so each frame needs only 2 scalar-engine
   evictions of [64, 1024] (ref half / emb half).
 - emb summed over t by identity-matmuls into PSUM; scores by 2x-scaled
   ones-matmuls over channel products; score transpose via PE; softmax on
   DVE; attn transposed back via the DMA XBAR (fp16); attn broadcast to
   (t,c) partitions by 0/1-matmuls; 1x1 conv as a K=512 matmul in 4
   K-tiles; LeakyReLU on the vector engine.
 - Tail drain of the last unit keeps the PE HAM-warm with filler matmuls.
"""

import sys
import os
import numpy as np

if "/opt/trn_rl_repo" not in sys.path:
    sys.path.insert(0, "/opt/trn_rl_repo")

B, T, C, H, W = 4, 8, 64, 128, 128
NCORES = 8
RPC = H // NCORES          # rows per core (16)
HS = RPC // 2              # rows per half-slab (8)
PX = HS * W                # pixels per half-slab (1024)
NCH = PX // 512            # 512-wide chunks per half-slab (2)

_CACHE: dict = {}

# attn transpose back to [t, px]: True = fp16 DMA XBAR, False = PE transposes
USE_DMA_T = False


# ----------------------------------------------------------------------------
# host-side input staging
# ----------------------------------------------------------------------------

def _stage_inputs(aligned_feat):
    """[B,T,C,H,W] -> per-core [B, 2, 128, T, 9, 130] dual-bank padded tiles."""
    af = np.ascontiguousarray(aligned_feat, dtype=np.float32)
    P = np.zeros((B, T, C, H + 2, W), np.float32)
    P[:, :, :, 1:-1, :] = af
    # rows[k, s, r] = 16k + 8s + r  (padded row index of slab row r)
    rows = (16 * np.arange(NCORES)[:, None, None]
            + HS * np.arange(2)[None, :, None]
            + np.arange(9)[None, None, :])
    A = np.zeros((NCORES, B, 2, 128, T, 9, 130), np.float32)
    # lo bank: channel c at slab row r; hi bank: channel c at slab row r+1
    Vlo = P[:, :, :, rows, :]          # [B,T,C,8,2,9,W]
    Vhi = P[:, :, :, rows + 1, :]
    A[:, :, :, :64, :, :, 1:129] = Vlo.transpose(3, 0, 4, 2, 1, 5, 6)
    A[:, :, :, 64:, :, :, 1:129] = Vhi.transpose(3, 0, 4, 2, 1, 5, 6)
    return A


def _make_consts(w1, b1, w2, b2, wf, bf):
    w1 = np.asarray(w1, np.float32); w2 = np.asarray(w2, np.float32)
    b2 = np.asarray(b2, np.float32)
    wf = np.asarray(wf, np.float32).reshape(C, T * C)
    bf = np.asarray(bf, np.float32)

    # conv lhsT: k = di*64 + cc (di in {0,1}); m = conv*64 + oc
    # (conv order swapped on odd frames so pair tiles assemble lane-locked)
    wta = np.zeros((128, 2, 3, 128), np.float32)
    wtb = np.zeros((128, 2, 3, 128), np.float32)
    for sw, (wa, wb) in enumerate([(w1, w2), (w2, w1)]):
        for dj in range(3):
            for di in range(2):
                wta[di * 64:(di + 1) * 64, sw, dj, :64] = wa[:, :, di, dj].T
                wta[di * 64:(di + 1) * 64, sw, dj, 64:] = wb[:, :, di, dj].T
            wtb[:64, sw, dj, :64] = wa[:, :, 2, dj].T
            wtb[:64, sw, dj, 64:] = wb[:, :, 2, dj].T
            wtb[64:, sw, dj, :] = wtb[:64, sw, dj, :]

    wsum = np.zeros((128, 128), np.float32)
    kk = np.arange(128)
    wsum[kk, kk % 64] = 1.0
    wsum[kk, kk % 64 + 64] = 1.0

    # score reduce; 2.0 = 1/TEMPERATURE folded in
    wsc = np.zeros((128, 4, 8), np.float32)
    for j in range(4):
        wsc[:64, j, 2 * j] = 2.0
        wsc[64:, j, 2 * j + 1] = 2.0

    we = np.zeros((8, 4, 128), np.float16 if USE_DMA_T else np.float32)
    for j in range(4):
        we[2 * j, j, :64] = 1.0
        we[2 * j + 1, j, 64:] = 1.0

    wwf = np.zeros((128, 4, 128), np.float32)
    for j in range(4):
        for p in range(2):
            blk = wf[:, (2 * j + p) * 64:(2 * j + p + 1) * 64].T  # [cc, oc]
            wwf[p * 64:(p + 1) * 64, j, :64] = blk
            wwf[p * 64:(p + 1) * 64, j, 64:] = blk

    bias = np.zeros((128, 2), np.float32)
    bias[:64, 0] = 8.0 * b2; bias[64:, 0] = 8.0 * b2
    bias[:64, 1] = bf; bias[64:, 1] = bf

    cc = {
        "wta": wta, "wtb": wtb, "wsum": wsum,
        "wsc": wsc, "we": we, "wwf": wwf, "bias": bias,
        "idt8": np.eye(8, dtype=np.float32),
    }
    if not USE_DMA_T:
        cc["idt128"] = np.eye(128, dtype=np.float32)
    return cc


CONST_SHAPES = {
    "wta": (128, 2, 3, 128), "wtb": (128, 2, 3, 128),
    "wsum": (128, 128), "wsc": (128, 4, 8), "we": (8, 4, 128),
    "wwf": (128, 4, 128), "bias": (128, 2), "idt8": (8, 8),
}
if not USE_DMA_T:
    CONST_SHAPES["idt128"] = (128, 128)


# ----------------------------------------------------------------------------
# kernel program
# ----------------------------------------------------------------------------

def build_nc():
    import concourse.bass as bass
    import concourse.tile as tile
    import concourse.mybir as mybir
    from concourse import bacc
    from contextlib import ExitStack

    f32 = mybir.dt.float32
    f32r = mybir.dt.float32r
    fp16 = mybir.dt.float16
    ACT = mybir.ActivationFunctionType
    ALU = mybir.AluOpType
    AX = mybir.AxisListType

    nc = bacc.Bacc("TRN2", target_bir_lowering=False, debug=False)

    CDT = {"wta": f32r, "wtb": f32r, "wsum": f32r,
           "wsc": f32r, "we": fp16 if USE_DMA_T else f32r, "wwf": f32r,
           "bias": f32, "idt8": f32, "idt128": f32}
    x = nc.dram_tensor("x", [B, 2, 128, T, 9, 130], f32r,
                       kind="ExternalInput").ap()
    cst = {n: nc.dram_tensor(n, list(s), CDT[n], kind="ExternalInput").ap()
           for n, s in CONST_SHAPES.items()}
    out = nc.dram_tensor("out", [B, 128, HS, W], f32, kind="ExternalOutput").ap()

    with tile.TileContext(nc) as tc, ExitStack() as ctx:
        cp = ctx.enter_context(tc.tile_pool(name="const", bufs=1))
        wup_f = cp.tile([128, 512], f32, tag="wup_f")
        nc.gpsimd.memset(wup_f[:], 1.0)
        consts = {}

        def load_consts(names):
            for n in names:
                s = CONST_SHAPES[n]
                t = cp.tile(list(s), CDT[n], tag=n, name=f"c_{n}")
                nc.sync.dma_start(t[:], cst[n][:])
                consts[n] = t

        xx_pool = ctx.enter_context(tc.tile_pool(name="xx", bufs=5))
        ref_pool = ctx.enter_context(tc.tile_pool(name="ref", bufs=4))
        emb_pool = ctx.enter_context(tc.tile_pool(name="emb", bufs=2))
        sml_pool = ctx.enter_context(tc.tile_pool(name="sml", bufs=2))
        prod_pool = ctx.enter_context(tc.tile_pool(name="prodp", bufs=4))
        out_pool = ctx.enter_context(tc.tile_pool(name="outb", bufs=2))
        # PSUM: ps1 = 2 tiles x 2 banks (conv cps, double-buffered frames);
        # pss = 1 tile x 2 banks (emb sums / scps, unit-cycled);
        # ps3 = 2 tiles x 1 bank (psT / abc rotation).
        ps1 = ctx.enter_context(tc.tile_pool(name="ps1", bufs=2, space="PSUM"))
        pss = ctx.enter_context(tc.tile_pool(name="pss", bufs=1, space="PSUM"))
        ps3 = ctx.enter_context(tc.tile_pool(name="ps3", bufs=2, space="PSUM"))

        state = {}

        def r(ap):
            return ap.bitcast(f32r)

        def load_xx(u):
            b, s = divmod(u, 2)
            xxh = []
            for hf in range(2):
                xt_ = xx_pool.tile([128, 4, 9, 130], f32r, tag="xx",
                                   name=f"xx{u}_{hf}")
                nc.sync.dma_start(xt_[:], x[b, s, :, 4 * hf:4 * (hf + 1)])
                xxh.append(xt_)
            return xxh

        def emit_a(u, inject, pre=None):
            """Conv/scores phase for unit u; inject[t] () emitted after
            frame t's convs (PE-stream interleaving for unit u-1)."""
            b, s = divmod(u, 2)
            xxh = load_xx(u) if pre is None else pre

            def xx(t):
                return xxh[t // 4][:, t % 4]

            sums = pss.tile([128, NCH, 512], f32, tag="pss", name=f"sum{u}")
            refs = []
            embs = []
            for t in range(T):
                j, sw = divmod(t, 2)
                if sw == 0:
                    refj = ref_pool.tile([128, NCH, 512], f32, tag="embref",
                                         name=f"ref{u}_{j}")
                    embj = emb_pool.tile([128, NCH, 512], f32r, tag="emb",
                                         name=f"emb{u}_{j}")
                    refs.append(refj)
                    embs.append(embj)
                refj, embj = refs[j], embs[j]
                cps = ps1.tile([128, NCH, 512], f32, tag="cps",
                               name=f"cps{u}_{t}")
                for dj in range(3):
                    for ch in range(NCH):
                        nc.tensor.matmul(
                            cps[:, ch], consts['wta'][:, sw, dj, :],
                            xx(t)[:, 4 * ch:4 * ch + 4, dj:dj + 128],
                            start=(dj == 0), stop=False)
                for dj in range(3):
                    nc.tensor.matmul(
                        cps[:, 0], consts['wtb'][0:64, sw, dj, :],
                        xx(t)[0:64, 2:6, dj:dj + 128],
                        start=False, stop=(dj == 2))
                    nc.tensor.matmul(
                        cps[:, 1], consts['wtb'][64:128, sw, dj, :],
                        xx(t)[64:128, 5:9, dj:dj + 128],
                        start=False, stop=(dj == 2))
                if sw == 0:
                    nc.scalar.activation(refj[0:64], cps[0:64], ACT.Copy)
                    nc.scalar.activation(embj[64:128], cps[64:128], ACT.Copy)
                else:
                    nc.scalar.activation(embj[0:64], cps[0:64], ACT.Copy)
                    nc.scalar.activation(refj[64:128], cps[64:128], ACT.Copy)
                if sw == 1:
                    for ch in range(NCH):
                        nc.tensor.matmul(sums[:, ch], consts['wsum'][:],
                                         embj[:, ch, :],
                                         start=(j == 0), stop=(j == 3))
                fn = inject.get(t)
                if fn:
                    fn()

            esum = sml_pool.tile([128, NCH, 512], f32, tag="esum",
                                 name=f"esum{u}")
            nc.scalar.activation(esum[:], sums[:], ACT.Identity,
                                 bias=consts['bias'][:, 0:1])
            scps = pss.tile([8, NCH, 512], f32, tag="pss", name=f"scps{u}")
            prods = []
            for j in range(4):
                prodj = prod_pool.tile([128, NCH, 512], f32r, tag="prod",
                                       name=f"prod{u}_{j}")
                nc.vector.tensor_mul(prodj[:].opt(), refs[j][:].opt(),
                                     esum[:].opt())
                prods.append(prodj)
            for j in range(4):
                for ch in range(NCH):
                    nc.tensor.matmul(scps[:, ch], consts['wsc'][:, j, :],
                                     prods[j][:, ch, :],
                                     start=(j == 0), stop=(j == 3))
            state[u] = {"xxh": xxh, "scps": scps}

        def filler(i):
            """One HAM-warming matmul on a scratch PSUM bank."""
            fps = ps3.tile([128, 4, 128], f32, tag="ps3", name=f"fill{i}",
                           bufs=2)
            nc.tensor.matmul(fps[:].opt(), r(wup_f[:, 0:128]), r(wup_f[:]),
                             start=True, stop=True)

        def b_pieces(u, tail=False):
            """B-phase for unit u as injectable pieces keyed by slot."""
            b, s = divmod(u, 2)
            st = state[u]

            def xx(t):
                return st["xxh"][t // 4][:, t % 4]

            def p_trans():
                scs = sml_pool.tile([8, NCH, 512], f32, tag="scs",
                                    name=f"scs{u}")
                nc.scalar.activation(scs[:], st["scps"][:], ACT.Copy)
                psT = ps3.tile([128, HS, 8], f32, tag="ps3", name=f"psT{u}")
                for h in range(HS):
                    nc.tensor.transpose(
                        psT[:, h, :],
                        scs[:].opt()[:, 128 * h:128 * (h + 1)], consts['idt8'][:])
                st["psT"] = psT

            def p_soft():
                from concourse.bass import broadcast_tensor_aps
                psT = st["psT"]
                nm = sml_pool.tile([128, HS, 1], f32, tag="nm",
                                   name=f"nm{u}")
                nc.vector.tensor_reduce(nm[:, :, 0], psT[:], axis=AX.X,
                                        op=ALU.max)
                sm = sml_pool.tile([128, HS, 8], f32, tag="sm",
                                   name=f"sm{u}")
                a0, a1 = broadcast_tensor_aps(psT[:], nm[:])
                nc.vector.tensor_tensor(sm[:], a0, a1, op=ALU.subtract)
                et = sml_pool.tile([128, HS, 8], f32, tag="et",
                                   name=f"et{u}")
                nc.scalar.activation(et[:].opt(), sm[:].opt(), ACT.Exp)
                den = sml_pool.tile([128, HS, 1], f32, tag="den",
                                    name=f"den{u}")
                nc.vector.tensor_reduce(den[:, :, 0], et[:], axis=AX.X,
                                        op=ALU.add)
                rec = sml_pool.tile([128, HS, 1], f32, tag="rec",
                                    name=f"rec{u}")
                nc.vector.reciprocal(rec[:], den[:])
                if USE_DMA_T:
                    attnT = sml_pool.tile([128, HS, 16], fp16, tag="attnT",
                                          name=f"attnT{u}")
                    # cols 8:16 transpose to attn partitions 8:16 (never
                    # read); zero them so the XBAR DMA doesn't read
                    # uninitialized SBUF
                    nc.gpsimd.memset(attnT[:, :, 8:16], 0.0)
                    b0, b1_ = broadcast_tensor_aps(et[:], rec[:])
                    nc.vector.tensor_tensor(attnT[:, :, 0:8], b0, b1_,
                                            op=ALU.mult)
                else:
                    attnT = sml_pool.tile([128, HS, 8], f32, tag="attnT",
                                          name=f"attnT{u}")
                    b0, b1_ = broadcast_tensor_aps(et[:], rec[:])
                    nc.vector.tensor_tensor(attnT[:], b0, b1_, op=ALU.mult)
                st["attnT"] = attnT

            def p_btrans():
                if USE_DMA_T:
                    attn = sml_pool.tile([16, HS, 128], fp16, tag="attn",
                                         name=f"attn{u}")
                    nc.sync.dma_start(attn[:], st["attnT"][:], transpose=True)
                    st["attn"] = attn
                    return
                attnT = st["attnT"]
                attn_ps = [ps3.tile([8, 512], f32, tag="ps3",
                                    name=f"attnps{u}_{ch}")
                           for ch in range(NCH)]
                for h in range(HS):
                    nc.tensor.transpose(
                        attn_ps[h // 4][:, 128 * (h % 4):128 * (h % 4 + 1)],
                        attnT[:, h, :], consts['idt128'][:])
                attn = sml_pool.tile([8, HS, 128], f32r, tag="attn",
                                     name=f"attn{u}")
                for ch in range(NCH):
                    nc.scalar.activation(attn[0:8, 4 * ch:4 * (ch + 1), :],
                                         attn_ps[ch][:], ACT.Copy)
                st["attn"] = attn

            def p_wj(j):
                def fn():
                    attn = st["attn"]
                    if tail and j == 0:
                        # conv stream is over; ps1 banks are free for the
                        # 1x1-conv accumulator so wwf can chase the wt muls
                        st["ops"] = ps1.tile([128, NCH, 512], f32, tag="cps",
                                             name=f"ops{u}")
                    wts = st.setdefault("wts", [])
                    for ch in range(NCH):
                        abc = ps3.tile([128, 4, 128], f32, tag="ps3",
                                       name=f"abc{u}_{j}_{ch}")
                        nc.tensor.matmul(abc[:].opt(), consts['we'][:, j, :],
                                         attn[0:8, 4 * ch:4 * (ch + 1), :].opt(),
                                         start=True, stop=True)
                        wt = sml_pool.tile([128, 4, 128], f32r, tag="wt",
                                           bufs=5, name=f"wt{u}_{j}_{ch}")
                        nc.vector.tensor_mul(
                            wt[0:64],
                            xx(2 * j)[0:64, 1 + 4 * ch:5 + 4 * ch, 1:129]
                            .bitcast(f32), abc[0:64])
                        nc.vector.tensor_mul(
                            wt[64:128],
                            xx(2 * j + 1)[64:128, 4 * ch:4 + 4 * ch, 1:129]
                            .bitcast(f32), abc[64:128])
                        wts.append(wt)
                        if tail:
                            nc.tensor.matmul(st["ops"][:, ch],
                                             consts['wwf'][:, j, :],
                                             wt[:].opt(),
                                             start=(j == 0), stop=(j == 3))
                return fn

            def p_wf():
                if not tail:
                    wts = st["wts"]
                    st["ops"] = ps1.tile([128, NCH, 512], f32, tag="cps",
                                         name=f"ops{u}")
                    for j in range(4):
                        for ch in range(NCH):
                            nc.tensor.matmul(st["ops"][:, ch],
                                             consts['wwf'][:, j, :],
                                             wts[2 * j + ch][:].opt(),
                                             start=(j == 0), stop=(j == 3))
                ops = st["ops"]
                outbuf = state.get(("ob", b))
                if outbuf is None:
                    outbuf = out_pool.tile([128, HS, W], f32, tag="outbuf",
                                           name=f"outbuf{b}")
                    state[("ob", b)] = outbuf
                yb = sml_pool.tile([128, NCH, 512], f32, tag="yb",
                                   name=f"yb{u}")
                nc.scalar.activation(
                    yb[64 * s:64 * (s + 1)],
                    ops[64 * s:64 * (s + 1)],
                    ACT.Identity, bias=consts['bias'][64 * s:64 * (s + 1), 1:2])
                nc.vector.scalar_tensor_tensor(
                    outbuf[64 * s:64 * (s + 1)].opt(),
                    yb[64 * s:64 * (s + 1)].opt(), 0.1,
                    yb[64 * s:64 * (s + 1)].opt(),
                    op0=ALU.mult, op1=ALU.max)
                nc.sync.dma_start(out[b, 64 * s:64 * (s + 1)],
                                  outbuf[64 * s:64 * (s + 1)])
                del state[u]
                if s == 1:
                    del state[("ob", b)]

            return {0: p_trans, 1: p_soft, 2: p_btrans,
                    3: p_wj(0), 4: p_wj(1), 5: p_wj(2), 6: p_wj(3),
                    7: p_wf}

        # --- startup: weights first, then frame 0 of unit 0, warmup MMs ---
        load_consts(["wta"])
        b0_, s0_ = 0, 0
        xx0 = xx_pool.tile([128, 4, 9, 130], f32r, tag="xx", name="xx0_0")
        nc.sync.dma_start(xx0[:, 0], x[b0_, s0_, :, 0])
        load_consts(["wtb"])
        for f in range(1, 4):
            nc.sync.dma_start(xx0[:, f], x[b0_, s0_, :, f])
        xx1 = xx_pool.tile([128, 4, 9, 130], f32r, tag="xx", name="xx0_1")
        for f in range(4):
            nc.sync.dma_start(xx1[:, f], x[b0_, s0_, :, 4 + f])
        load_consts([n for n in CONST_SHAPES if n not in ("wta", "wtb")])

        wup = ps1.tile([128, NCH, 512], f32, tag="cps", name="wup")
        for i in range(16):
            nc.tensor.matmul(wup[:, 0], r(wup_f[:, 0:128]), r(wup_f[:]),
                             start=True, stop=True)

        emit_a(0, {}, [xx0, xx1])
        for u in range(1, 8):
            emit_a(u, b_pieces(u - 1))
        # drain last unit; fillers keep the PE HAM-warm through the chain
        tailp = b_pieces(7, tail=True)
        for k in range(8):
            tailp[k]()
            if k < 7:
                nfill = 7 if k < 3 else 3
                for i in range(nfill):
                    filler(10 * k + i)

    nc.compile()
    return nc


# ----------------------------------------------------------------------------
# entry point
# ----------------------------------------------------------------------------

def kernel(aligned_feat, w1, b1, w2, b2, wf, bf):
    from concourse import bass_utils

    if "nc" not in _CACHE:
        _CACHE["nc"] = build_nc()
    nc = _CACHE["nc"]

    A = _stage_inputs(aligned_feat)
    consts = _make_consts(w1, b1, w2, b2, wf, bf)
    in_maps = [{"x": A[k], **consts} for k in range(NCORES)]

    res = bass_utils.run_bass_kernel_spmd(nc, in_maps, core_ids=list(range(NCORES)))
    outs = [res.results[k]["out"] for k in range(NCORES)]  # [B,128,HS,W]

    full = np.empty((B, C, H, W), np.float32)
    for k in range(NCORES):
        o = outs[k]
        for s in range(2):
            full[:, :, RPC * k + HS * s: RPC * k + HS * (s + 1), :] = \
                o[:, 64 * s:64 * (s + 1), :, :]
    return full


# revision 32
# speedup vs baseline: 235682.6377x; 1.0198x over previous
"""BGFusionBlock Trainium2 kernel (Bass/Tile, 8 NeuronCores, SPMD).

Shapes: aligned_feat [4, 8, 64, 128, 128] f32, w1/w2 [64, 64, 3, 3],
b1/b2 [64], wf [64, 512, 1, 1], bf [64].  Output [4, 64, 128, 128] f32.

Math:
  emb     = conv3x3(x, w2)   per frame           (biases: b2 enters via esum)
  emb_ref = conv3x3(x, w1)   per frame           (b1 cancels in the softmax)
  scores[b,t,p] = <emb_ref[b,t,:,p], sum_j emb[b,j,:,p] + 8*b2>
  attn = softmax(2*scores, axis=t)
  out  = leaky_relu(conv1x1(aligned_feat * attn, wf) + bf, 0.1)

Distribution: shard H across the 8 cores (16 rows each, 1-row halo baked
into the per-core input by the host).  All compute is core-local.

Per-core plan (each (b, half-slab of 8 rows) is one unit, px = 8*128 = 1024):
 - x staged as [128p, 8t, 9r, 130c] f32r tiles: partitions 0:64 = channel c
   at slab row r, partitions 64:128 = channel c at slab row r+1.  K=128
   matmuls cover vertical tap pairs (0,dj)+(1,dj); M=128 packs BOTH convs
   (w1|w2, swapped on odd frames).
 - The di=2 tail is packed via a second fp16 tensor xt [128p, 8t, 8r, 130c]:
   partitions 0:64 = row r+2 (col c-1), partitions 64:128 = row r+2 (col c),
   so ONE K=128 matmul covers taps (2,0)+(2,1) and one K=64 matmul (hi bank)
   covers (2,2).  5 matmuls per 512-px chunk per frame (vs 6 unpacked).
 - cps PSUM tiles span 2 banks so each frame needs only 2 scalar-engine
   evictions of [64, 1024] (ref half / emb half).
 - emb summed over t by identity-matmuls into PSUM; scores by 2x-scaled
   ones-matmuls over channel products; score transpose via PE; softmax on
   DVE; attn transposed back via the DMA XBAR (fp16); attn broadcast to
   (t,c) partitions by 0/1-matmuls; 1x1 conv as a K=512 matmul in 4
   K-tiles; LeakyReLU on the vector engine.
 - Tail drain of the last unit keeps the PE HAM-warm with filler matmuls.
"""

import sys
import os
import numpy as np

if "/opt/trn_rl_repo" not in sys.path:
    sys.path.insert(0, "/opt/trn_rl_repo")

B, T, C, H, W = 4, 8, 64, 128, 128
NCORES = 8
RPC = H // NCORES          # rows per core (16)
HS = RPC // 2              # rows per half-slab (8)
PX = HS * W                # pixels per half-slab (1024)
NCH = PX // 512            # 512-wide chunks per half-slab (2)

_CACHE: dict = {}

# attn transpose back to [t, px]: True = fp16 DMA XBAR, False = PE transposes
USE_DMA_T = False


# ----------------------------------------------------------------------------
# host-side input staging
# ----------------------------------------------------------------------------

def _stage_inputs(aligned_feat):
    """[B,T,C,H,W] -> per-core [B, 2, 128, T, 9, 130] dual-bank padded tiles."""
    af = np.ascontiguousarray(aligned_feat, dtype=np.float32)
    P = np.zeros((B, T, C, H + 2, W), np.float32)
    P[:, :, :, 1:-1, :] = af
    # rows[k, s, r] = 16k + 8s + r  (padded row index of slab row r)
    rows = (16 * np.arange(NCORES)[:, None, None]
            + HS * np.arange(2)[None, :, None]
            + np.arange(9)[None, None, :])
    A = np.zeros((NCORES, B, 2, 128, T, 9, 130), np.float32)
    # lo bank: channel c at slab row r; hi bank: channel c at slab row r+1
    Vlo = P[:, :, :, rows, :]          # [B,T,C,8,2,9,W]
    Vhi = P[:, :, :, rows + 1, :]
    A[:, :, :, :64, :, :, 1:129] = Vlo.transpose(3, 0, 4, 2, 1, 5, 6)
    A[:, :, :, 64:, :, :, 1:129] = Vhi.transpose(3, 0, 4, 2, 1, 5, 6)
    return A


def _make_consts(w1, b1, w2, b2, wf, bf):
    w1 = np.asarray(w1, np.float32); w2 = np.asarray(w2, np.float32)
    b2 = np.asarray(b2, np.float32)
    wf = np.asarray(wf, np.float32).reshape(C, T * C)
    bf = np.asarray(bf, np.float32)

    # conv lhsT: k = di*64 + cc (di in {0,1}); m = conv*64 + oc
    # (conv order swapped on odd frames so pair tiles assemble lane-locked)
    wta = np.zeros((128, 2, 3, 128), np.float32)
    wtb = np.zeros((128, 2, 3, 128), np.float32)
    for sw, (wa, wb) in enumerate([(w1, w2), (w2, w1)]):
        for dj in range(3):
            for di in range(2):
                wta[di * 64:(di + 1) * 64, sw, dj, :64] = wa[:, :, di, dj].T
                wta[di * 64:(di + 1) * 64, sw, dj, 64:] = wb[:, :, di, dj].T
            wtb[:64, sw, dj, :64] = wa[:, :, 2, dj].T
            wtb[:64, sw, dj, 64:] = wb[:, :, 2, dj].T
            wtb[64:, sw, dj, :] = wtb[:64, sw, dj, :]

    wsum = np.zeros((128, 128), np.float32)
    kk = np.arange(128)
    wsum[kk, kk % 64] = 1.0
    wsum[kk, kk % 64 + 64] = 1.0

    # score reduce; 2.0 = 1/TEMPERATURE folded in
    wsc = np.zeros((128, 4, 8), np.float32)
    for j in range(4):
        wsc[:64, j, 2 * j] = 2.0
        wsc[64:, j, 2 * j + 1] = 2.0

    we = np.zeros((8, 4, 128), np.float16 if USE_DMA_T else np.float32)
    for j in range(4):
        we[2 * j, j, :64] = 1.0
        we[2 * j + 1, j, 64:] = 1.0

    wwf = np.zeros((128, 4, 128), np.float32)
    for j in range(4):
        for p in range(2):
            blk = wf[:, (2 * j + p) * 64:(2 * j + p + 1) * 64].T  # [cc, oc]
            wwf[p * 64:(p + 1) * 64, j, :64] = blk
            wwf[p * 64:(p + 1) * 64, j, 64:] = blk

    bias = np.zeros((128, 2), np.float32)
    bias[:64, 0] = 8.0 * b2; bias[64:, 0] = 8.0 * b2
    bias[:64, 1] = bf; bias[64:, 1] = bf

    cc = {
        "wta": wta, "wtb": wtb, "wsum": wsum,
        "wsc": wsc, "we": we, "wwf": wwf, "bias": bias,
        "idt8": np.eye(8, dtype=np.float32),
    }
    if not USE_DMA_T:
        cc["idt128"] = np.eye(128, dtype=np.float32)
    return cc


CONST_SHAPES = {
    "wta": (128, 2, 3, 128), "wtb": (128, 2, 3, 128),
    "wsum": (128, 128), "wsc": (128, 4, 8), "we": (8, 4, 128),
    "wwf": (128, 4, 128), "bias": (128, 2), "idt8": (8, 8),
}
if not USE_DMA_T:
    CONST_SHAPES["idt128"] = (128, 128)


# ----------------------------------------------------------------------------
# kernel program
# ----------------------------------------------------------------------------

def build_nc():
    import concourse.bass as bass
    import concourse.tile as tile
    import concourse.mybir as mybir
    from concourse import bacc
    from contextlib import ExitStack

    f32 = mybir.dt.float32
    f32r = mybir.dt.float32r
    fp16 = mybir.dt.float16
    ACT = mybir.ActivationFunctionType
    ALU = mybir.AluOpType
    AX = mybir.AxisListType

    nc = bacc.Bacc("TRN2", target_bir_lowering=False, debug=False)

    CDT = {"wta": f32r, "wtb": f32r, "wsum": f32r,
           "wsc": f32r, "we": fp16 if USE_DMA_T else f32r, "wwf": f32r,
           "bias": f32, "idt8": f32, "idt128": f32}
    x = nc.dram_tensor("x", [B, 2, 128, T, 9, 130], f32r,
                       kind="ExternalInput").ap()
    cst = {n: nc.dram_tensor(n, list(s), CDT[n], kind="ExternalInput").ap()
           for n, s in CONST_SHAPES.items()}
    out = nc.dram_tensor("out", [B, 128, HS, W], f32, kind="ExternalOutput").ap()

    with tile.TileContext(nc) as tc, ExitStack() as ctx:
        cp = ctx.enter_context(tc.tile_pool(name="const", bufs=1))
        wup_f = cp.tile([128, 512], f32, tag="wup_f")
        nc.gpsimd.memset(wup_f[:], 1.0)
        consts = {}

        def load_consts(names):
            for n in names:
                s = CONST_SHAPES[n]
                t = cp.tile(list(s), CDT[n], tag=n, name=f"c_{n}")
                nc.sync.dma_start(t[:], cst[n][:])
                consts[n] = t

        xx_pool = ctx.enter_context(tc.tile_pool(name="xx", bufs=5))
        ref_pool = ctx.enter_context(tc.tile_pool(name="ref", bufs=4))
        emb_pool = ctx.enter_context(tc.tile_pool(name="emb", bufs=2))
        sml_pool = ctx.enter_context(tc.tile_pool(name="sml", bufs=2))
        prod_pool = ctx.enter_context(tc.tile_pool(name="prodp", bufs=4))
        out_pool = ctx.enter_context(tc.tile_pool(name="outb", bufs=2))
        # PSUM: ps1 = 2 tiles x 2 banks (conv cps, double-buffered frames);
        # pss = 1 tile x 2 banks (emb sums / scps, unit-cycled);
        # ps3 = 2 tiles x 1 bank (psT / abc rotation).
        ps1 = ctx.enter_context(tc.tile_pool(name="ps1", bufs=2, space="PSUM"))
        pss = ctx.enter_context(tc.tile_pool(name="pss", bufs=1, space="PSUM"))
        ps3 = ctx.enter_context(tc.tile_pool(name="ps3", bufs=2, space="PSUM"))

        state = {}

        def r(ap):
            return ap.bitcast(f32r)

        def load_xx(u):
            b, s = divmod(u, 2)
            xxh = []
            for hf in range(2):
                xt_ = xx_pool.tile([128, 4, 9, 130], f32r, tag="xx",
                                   name=f"xx{u}_{hf}")
                nc.sync.dma_start(xt_[:], x[b, s, :, 4 * hf:4 * (hf + 1)])
                xxh.append(xt_)
            return xxh

        def emit_a(u, inject, pre=None, post=None):
            """Conv/scores phase for unit u; inject[t] () emitted after
            frame t's convs (PE-stream interleaving for unit u-1)."""
            b, s = divmod(u, 2)
            xxh = load_xx(u) if pre is None else pre

            def xx(t):
                return xxh[t // 4][:, t % 4]

            sums = pss.tile([128, NCH, 512], f32, tag="pss", name=f"sum{u}")
            refs = []
            embs = []
            for t in range(T):
                j, sw = divmod(t, 2)
                if sw == 0:
                    refj = ref_pool.tile([128, NCH, 512], f32, tag="embref",
                                         name=f"ref{u}_{j}")
                    embj = emb_pool.tile([128, NCH, 512], f32r, tag="emb",
                                         name=f"emb{u}_{j}")
                    refs.append(refj)
                    embs.append(embj)
                refj, embj = refs[j], embs[j]
                cps = ps1.tile([128, NCH, 512], f32, tag="cps",
                               name=f"cps{u}_{t}")
                for dj in range(3):
                    for ch in range(NCH):
                        nc.tensor.matmul(
                            cps[:, ch], consts['wta'][:, sw, dj, :],
                            xx(t)[:, 4 * ch:4 * ch + 4, dj:dj + 128],
                            start=(dj == 0), stop=False)
                for dj in range(3):
                    nc.tensor.matmul(
                        cps[:, 0], consts['wtb'][0:64, sw, dj, :],
                        xx(t)[0:64, 2:6, dj:dj + 128],
                        start=False, stop=(dj == 2))
                    nc.tensor.matmul(
                        cps[:, 1], consts['wtb'][64:128, sw, dj, :],
                        xx(t)[64:128, 5:9, dj:dj + 128],
                        start=False, stop=(dj == 2))
                if sw == 0:
                    nc.scalar.activation(refj[0:64], cps[0:64], ACT.Copy)
                    nc.scalar.activation(embj[64:128], cps[64:128], ACT.Copy)
                else:
                    nc.scalar.activation(embj[0:64], cps[0:64], ACT.Copy)
                    nc.scalar.activation(refj[64:128], cps[64:128], ACT.Copy)
                if sw == 1:
                    for ch in range(NCH):
                        nc.tensor.matmul(sums[:, ch], consts['wsum'][:],
                                         embj[:, ch, :],
                                         start=(j == 0), stop=(j == 3))
                fn = inject.get(t)
                if fn:
                    fn()

            if post:
                post()
            esum = sml_pool.tile([128, NCH, 512], f32, tag="esum",
                                 name=f"esum{u}")
            nc.scalar.activation(esum[:], sums[:], ACT.Identity,
                                 bias=consts['bias'][:, 0:1])
            scps = pss.tile([8, NCH, 512], f32, tag="pss", name=f"scps{u}")
            prods = []
            for j in range(4):
                prodj = prod_pool.tile([128, NCH, 512], f32r, tag="prod",
                                       name=f"prod{u}_{j}")
                nc.vector.tensor_mul(prodj[:].opt(), refs[j][:].opt(),
                                     esum[:].opt())
                prods.append(prodj)
            for j in range(4):
                for ch in range(NCH):
                    nc.tensor.matmul(scps[:, ch], consts['wsc'][:, j, :],
                                     prods[j][:, ch, :],
                                     start=(j == 0), stop=(j == 3))
            state[u] = {"xxh": xxh, "scps": scps}

        _fill_n = [0]

        def filler(pool, n=1):
            """HAM-warming matmuls on scratch PSUM banks of `pool`."""
            for _ in range(n):
                i = _fill_n[0]
                _fill_n[0] += 1
                fps = pool.tile([128, NCH, 512], f32,
                                tag="cps" if pool is ps1 else "pss",
                                name=f"fill{i}")
                nc.tensor.matmul(fps[:, 0], r(wup_f[:, 0:128]), r(wup_f[:]),
                                 start=True, stop=True)

        def b_pieces(u, tail=False):
            """B-phase for unit u as injectable pieces keyed by slot."""
            b, s = divmod(u, 2)
            st = state[u]

            def xx(t):
                return st["xxh"][t // 4][:, t % 4]

            def p_trans():
                scs = sml_pool.tile([8, NCH, 512], f32, tag="scs",
                                    name=f"scs{u}")
                nc.scalar.activation(scs[:], st["scps"][:], ACT.Copy)
                if tail:
                    filler(ps1, 3)
                psT = ps3.tile([128, HS, 8], f32, tag="ps3", name=f"psT{u}")
                for h in range(HS):
                    nc.tensor.transpose(
                        psT[:, h, :],
                        scs[:].opt()[:, 128 * h:128 * (h + 1)], consts['idt8'][:])
                st["psT"] = psT

            def p_soft():
                from concourse.bass import broadcast_tensor_aps
                psT = st["psT"]
                nm = sml_pool.tile([128, HS, 1], f32, tag="nm",
                                   name=f"nm{u}")
                nc.vector.tensor_reduce(nm[:, :, 0], psT[:], axis=AX.X,
                                        op=ALU.max)
                sm = sml_pool.tile([128, HS, 8], f32, tag="sm",
                                   name=f"sm{u}")
                a0, a1 = broadcast_tensor_aps(psT[:], nm[:])
                nc.vector.tensor_tensor(sm[:], a0, a1, op=ALU.subtract)
                et = sml_pool.tile([128, HS, 8], f32, tag="et",
                                   name=f"et{u}")
                nc.scalar.activation(et[:].opt(), sm[:].opt(), ACT.Exp)
                den = sml_pool.tile([128, HS, 1], f32, tag="den",
                                    name=f"den{u}")
                nc.vector.tensor_reduce(den[:, :, 0], et[:], axis=AX.X,
                                        op=ALU.add)
                rec = sml_pool.tile([128, HS, 1], f32, tag="rec",
                                    name=f"rec{u}")
                nc.vector.reciprocal(rec[:], den[:])
                if USE_DMA_T:
                    attnT = sml_pool.tile([128, HS, 16], fp16, tag="attnT",
                                          name=f"attnT{u}")
                    # cols 8:16 transpose to attn partitions 8:16 (never
                    # read); zero them so the XBAR DMA doesn't read
                    # uninitialized SBUF
                    nc.gpsimd.memset(attnT[:, :, 8:16], 0.0)
                    b0, b1_ = broadcast_tensor_aps(et[:], rec[:])
                    nc.vector.tensor_tensor(attnT[:, :, 0:8], b0, b1_,
                                            op=ALU.mult)
                else:
                    attnT = sml_pool.tile([128, HS, 8], f32, tag="attnT",
                                          name=f"attnT{u}")
                    b0, b1_ = broadcast_tensor_aps(et[:], rec[:])
                    nc.vector.tensor_tensor(attnT[:], b0, b1_, op=ALU.mult)
                st["attnT"] = attnT

            def p_btrans():
                if USE_DMA_T:
                    attn = sml_pool.tile([16, HS, 128], fp16, tag="attn",
                                         name=f"attn{u}")
                    nc.sync.dma_start(attn[:], st["attnT"][:], transpose=True)
                    st["attn"] = attn
                    return
                attnT = st["attnT"]
                attn_ps = [ps3.tile([8, 512], f32, tag="ps3",
                                    name=f"attnps{u}_{ch}")
                           for ch in range(NCH)]
                for h in range(HS):
                    nc.tensor.transpose(
                        attn_ps[h // 4][:, 128 * (h % 4):128 * (h % 4 + 1)],
                        attnT[:, h, :], consts['idt128'][:])
                attn = sml_pool.tile([8, HS, 128], f32r, tag="attn",
                                     name=f"attn{u}")
                for ch in range(NCH):
                    nc.scalar.activation(attn[0:8, 4 * ch:4 * (ch + 1), :],
                                         attn_ps[ch][:], ACT.Copy)
                st["attn"] = attn

            def t_abc(j):
                """abc matmuls for round j (and the ops alloc on round 0)."""
                attn = st["attn"]
                if j == 0:
                    # conv stream is over; ps1 banks are free for the
                    # 1x1-conv accumulator so wwf can chase the wt muls
                    st["ops"] = ps1.tile([128, NCH, 512], f32, tag="cps",
                                         name=f"ops{u}")
                abcs = st.setdefault("abcs", {})
                abcs[j] = []
                for ch in range(NCH):
                    abc = ps3.tile([128, 4, 128], f32, tag="ps3",
                                   name=f"abc{u}_{j}_{ch}")
                    nc.tensor.matmul(abc[:].opt(), consts['we'][:, j, :],
                                     attn[0:8, 4 * ch:4 * (ch + 1), :].opt(),
                                     start=True, stop=True)
                    abcs[j].append(abc)

            def t_wt(j):
                wtsj = st.setdefault("wtsj", {})
                wtsj[j] = []
                for ch in range(NCH):
                    abc = st["abcs"][j][ch]
                    wt = sml_pool.tile([128, 4, 128], f32r, tag="wt",
                                       bufs=5, name=f"wt{u}_{j}_{ch}")
                    nc.vector.tensor_mul(
                        wt[0:64],
                        xx(2 * j)[0:64, 1 + 4 * ch:5 + 4 * ch, 1:129]
                        .bitcast(f32), abc[0:64])
                    nc.vector.tensor_mul(
                        wt[64:128],
                        xx(2 * j + 1)[64:128, 4 * ch:4 + 4 * ch, 1:129]
                        .bitcast(f32), abc[64:128])
                    wtsj[j].append(wt)

            def t_wwf(j):
                for ch in range(NCH):
                    nc.tensor.matmul(st["ops"][:, ch], consts['wwf'][:, j, :],
                                     st["wtsj"][j][ch][:].opt(),
                                     start=(j == 0), stop=(j == 3))

            def p_wj(j):
                def fn():
                    attn = st["attn"]
                    wts = st.setdefault("wts", [])
                    for ch in range(NCH):
                        abc = ps3.tile([128, 4, 128], f32, tag="ps3",
                                       name=f"abc{u}_{j}_{ch}")
                        nc.tensor.matmul(abc[:].opt(), consts['we'][:, j, :],
                                         attn[0:8, 4 * ch:4 * (ch + 1), :].opt(),
                                         start=True, stop=True)
                        wt = sml_pool.tile([128, 4, 128], f32r, tag="wt",
                                           bufs=5, name=f"wt{u}_{j}_{ch}")
                        nc.vector.tensor_mul(
                            wt[0:64],
                            xx(2 * j)[0:64, 1 + 4 * ch:5 + 4 * ch, 1:129]
                            .bitcast(f32), abc[0:64])
                        nc.vector.tensor_mul(
                            wt[64:128],
                            xx(2 * j + 1)[64:128, 4 * ch:4 + 4 * ch, 1:129]
                            .bitcast(f32), abc[64:128])
                        wts.append(wt)
                return fn

            def p_wf():
                wts = st["wts"]
                st["ops"] = ps1.tile([128, NCH, 512], f32, tag="cps",
                                     name=f"ops{u}")
                for j in range(4):
                    for ch in range(NCH):
                        nc.tensor.matmul(st["ops"][:, ch],
                                         consts['wwf'][:, j, :],
                                         wts[2 * j + ch][:].opt(),
                                         start=(j == 0), stop=(j == 3))
                finish()

            def finish():
                ops = st["ops"]
                outbuf = state.get(("ob", b))
                if outbuf is None:
                    outbuf = out_pool.tile([128, HS, W], f32, tag="outbuf",
                                           name=f"outbuf{b}")
                    state[("ob", b)] = outbuf
                yb = sml_pool.tile([128, NCH, 512], f32, tag="yb",
                                   name=f"yb{u}")
                for ch in range(NCH):
                    nc.scalar.activation(
                        yb[64 * s:64 * (s + 1), ch],
                        ops[64 * s:64 * (s + 1), ch],
                        ACT.Identity,
                        bias=consts['bias'][64 * s:64 * (s + 1), 1:2])
                    nc.vector.scalar_tensor_tensor(
                        outbuf[64 * s:64 * (s + 1),
                               4 * ch:4 * (ch + 1)].opt(),
                        yb[64 * s:64 * (s + 1), ch].opt(), 0.1,
                        yb[64 * s:64 * (s + 1), ch].opt(),
                        op0=ALU.mult, op1=ALU.max)
                    nc.sync.dma_start(
                        out[b, 64 * s:64 * (s + 1), 4 * ch:4 * (ch + 1)],
                        outbuf[64 * s:64 * (s + 1), 4 * ch:4 * (ch + 1)])
                del state[u]
                if s == 1:
                    del state[("ob", b)]

            if tail:
                return {"trans": p_trans, "soft": p_soft, "btrans": p_btrans,
                        "abc": t_abc, "wt": t_wt, "wwf": t_wwf,
                        "finish": finish}
            return {0: p_trans, 1: p_soft, 2: p_btrans,
                    3: p_wj(0), 4: p_wj(1), 5: p_wj(2), 6: p_wj(3),
                    7: p_wf}

        # --- startup: weights first, then frame 0 of unit 0, warmup MMs ---
        load_consts(["wta"])
        b0_, s0_ = 0, 0
        xx0 = xx_pool.tile([128, 4, 9, 130], f32r, tag="xx", name="xx0_0")
        nc.sync.dma_start(xx0[:, 0], x[b0_, s0_, :, 0])
        load_consts(["wtb"])
        for f in range(1, 4):
            nc.sync.dma_start(xx0[:, f], x[b0_, s0_, :, f])
        xx1 = xx_pool.tile([128, 4, 9, 130], f32r, tag="xx", name="xx0_1")
        for f in range(4):
            nc.sync.dma_start(xx1[:, f], x[b0_, s0_, :, 4 + f])
        load_consts([n for n in CONST_SHAPES if n not in ("wta", "wtb")])

        wup = ps1.tile([128, NCH, 512], f32, tag="cps", name="wup")
        for i in range(16):
            nc.tensor.matmul(wup[:, 0], r(wup_f[:, 0:128]), r(wup_f[:]),
                             start=True, stop=True)

        emit_a(0, {}, [xx0, xx1])
        for u in range(1, 7):
            emit_a(u, b_pieces(u - 1))
        # fillers ride the PE through unit 7's esum/prods window so HAM
        # stays at K=8/8 into the drain
        emit_a(7, b_pieces(6), post=lambda: filler(ps1, 6))
        # drain last unit: abc -> wt -> wwf software-pipelined, fillers in
        # every PE hole (ps1 until `ops` is allocated, pss afterwards)
        tp = b_pieces(7, tail=True)
        tp["trans"]()
        filler(ps1, 6)
        tp["soft"]()
        filler(ps1, 6)
        tp["btrans"]()
        filler(ps1, 5)
        tp["abc"](0)
        tp["wt"](0)
        tp["abc"](1)
        for j in range(3):
            tp["wwf"](j)
            filler(pss, 2)
            tp["wt"](j + 1)
            if j < 2:
                tp["abc"](j + 2)
        tp["wwf"](3)
        tp["finish"]()

    nc.compile()
    return nc


# ----------------------------------------------------------------------------
# entry point
# ----------------------------------------------------------------------------

def kernel(aligned_feat, w1, b1, w2, b2, wf, bf):
    from concourse import bass_utils

    if "nc" not in _CACHE:
        _CACHE["nc"] = build_nc()
    nc = _CACHE["nc"]

    A = _stage_inputs(aligned_feat)
    consts = _make_consts(w1, b1, w2, b2, wf, bf)
    in_maps = [{"x": A[k], **consts} for k in range(NCORES)]

    res = bass_utils.run_bass_kernel_spmd(nc, in_maps, core_ids=list(range(NCORES)))
    outs = [res.results[k]["out"] for k in range(NCORES)]  # [B,128,HS,W]

    full = np.empty((B, C, H, W), np.float32)
    for k in range(NCORES):
        o = outs[k]
        for s in range(2):
            full[:, :, RPC * k + HS * s: RPC * k + HS * (s + 1), :] = \
                o[:, 64 * s:64 * (s + 1), :, :]
    return full


# revision 43
# speedup vs baseline: 241430.6898x; 1.0244x over previous
"""BGFusionBlock Trainium2 kernel (Bass/Tile, 8 NeuronCores, SPMD).

Shapes: aligned_feat [4, 8, 64, 128, 128] f32, w1/w2 [64, 64, 3, 3],
b1/b2 [64], wf [64, 512, 1, 1], bf [64].  Output [4, 64, 128, 128] f32.

Math:
  emb     = conv3x3(x, w2)   per frame           (biases: b2 enters via esum)
  emb_ref = conv3x3(x, w1)   per frame           (b1 cancels in the softmax)
  scores[b,t,p] = <emb_ref[b,t,:,p], sum_j emb[b,j,:,p] + 8*b2>
  attn = softmax(2*scores, axis=t)
  out  = leaky_relu(conv1x1(aligned_feat * attn, wf) + bf, 0.1)

Distribution: shard H across the 8 cores (16 rows each, 1-row halo baked
into the per-core input by the host).  All compute is core-local.

Per-core plan (each (b, half-slab of 8 rows) is one unit, px = 8*128 = 1024):
 - x staged as [128p, 8t, 9r, 130c] f32r tiles: partitions 0:64 = channel c
   at slab row r, partitions 64:128 = channel c at slab row r+1.  K=128
   matmuls cover vertical tap pairs (0,dj)+(1,dj); M=128 packs BOTH convs
   (w1|w2, swapped on odd frames).
 - The di=2 tail is packed via a second fp16 tensor xt [128p, 8t, 8r, 130c]:
   partitions 0:64 = row r+2 (col c-1), partitions 64:128 = row r+2 (col c),
   so ONE K=128 matmul covers taps (2,0)+(2,1) and one K=64 matmul (hi bank)
   covers (2,2).  5 matmuls per 512-px chunk per frame (vs 6 unpacked).
 - cps PSUM tiles span 2 banks so each frame needs only 2 scalar-engine
   evictions of [64, 1024] (ref half / emb half).
 - emb summed over t by identity-matmuls into PSUM; scores by 2x-scaled
   ones-matmuls over channel products; score transpose via PE; softmax on
   DVE; attn transposed back via the DMA XBAR (fp16); attn broadcast to
   (t,c) partitions by 0/1-matmuls; 1x1 conv as a K=512 matmul in 4
   K-tiles; LeakyReLU on the vector engine.
 - Tail drain of the last unit keeps the PE HAM-warm with filler matmuls.
"""

import sys
import os
import numpy as np

if "/opt/trn_rl_repo" not in sys.path:
    sys.path.insert(0, "/opt/trn_rl_repo")

B, T, C, H, W = 4, 8, 64, 128, 128
NCORES = 8
RPC = H // NCORES          # rows per core (16)
HS = RPC // 2              # rows per half-slab (8)
PX = HS * W                # pixels per half-slab (1024)
NCH = PX // 512            # 512-wide chunks per half-slab (2)

_CACHE: dict = {}


# ----------------------------------------------------------------------------
# host-side input staging
# ----------------------------------------------------------------------------

def _stage_inputs(aligned_feat):
    """[B,T,C,H,W] -> per-core [B, 2, 128, T, 9, 130] dual-bank padded tiles."""
    af = np.ascontiguousarray(aligned_feat, dtype=np.float32)
    P = np.zeros((B, T, C, H + 2, W), np.float32)
    P[:, :, :, 1:-1, :] = af
    # rows[k, s, r] = 16k + 8s + r  (padded row index of slab row r)
    rows = (16 * np.arange(NCORES)[:, None, None]
            + HS * np.arange(2)[None, :, None]
            + np.arange(9)[None, None, :])
    A = np.zeros((NCORES, B, 2, 128, T, 9, 130), np.float32)
    # lo bank: channel c at slab row r; hi bank: channel c at slab row r+1
    Vlo = P[:, :, :, rows, :]          # [B,T,C,8,2,9,W]
    Vhi = P[:, :, :, rows + 1, :]
    A[:, :, :, :64, :, :, 1:129] = Vlo.transpose(3, 0, 4, 2, 1, 5, 6)
    A[:, :, :, 64:, :, :, 1:129] = Vhi.transpose(3, 0, 4, 2, 1, 5, 6)
    return A


def _make_consts(w1, b1, w2, b2, wf, bf):
    w1 = np.asarray(w1, np.float32); w2 = np.asarray(w2, np.float32)
    b2 = np.asarray(b2, np.float32)
    wf = np.asarray(wf, np.float32).reshape(C, T * C)
    bf = np.asarray(bf, np.float32)

    # conv lhsT: k = di*64 + cc (di in {0,1}); m = conv*64 + oc
    # (conv order swapped on odd frames so pair tiles assemble lane-locked)
    wta = np.zeros((128, 2, 3, 128), np.float32)
    wtb = np.zeros((128, 2, 3, 128), np.float32)
    for sw, (wa, wb) in enumerate([(w1, w2), (w2, w1)]):
        for dj in range(3):
            for di in range(2):
                wta[di * 64:(di + 1) * 64, sw, dj, :64] = wa[:, :, di, dj].T
                wta[di * 64:(di + 1) * 64, sw, dj, 64:] = wb[:, :, di, dj].T
            wtb[:64, sw, dj, :64] = wa[:, :, 2, dj].T
            wtb[:64, sw, dj, 64:] = wb[:, :, 2, dj].T
            wtb[64:, sw, dj, :] = wtb[:64, sw, dj, :]

    wsum = np.zeros((128, 128), np.float32)
    kk = np.arange(128)
    wsum[kk, kk % 64] = 1.0
    wsum[kk, kk % 64 + 64] = 1.0

    # score reduce; 2.0 = 1/TEMPERATURE folded in
    wsc = np.zeros((128, 4, 8), np.float32)
    for j in range(4):
        wsc[:64, j, 2 * j] = 2.0
        wsc[64:, j, 2 * j + 1] = 2.0

    import ml_dtypes
    we = np.zeros((8, 4, 128), np.float32)
    for j in range(4):
        we[2 * j, j, :64] = 1.0
        we[2 * j + 1, j, 64:] = 1.0
    web = we.astype(ml_dtypes.bfloat16)

    wwf = np.zeros((128, 4, 128), np.float32)
    for j in range(4):
        for p in range(2):
            blk = wf[:, (2 * j + p) * 64:(2 * j + p + 1) * 64].T  # [cc, oc]
            wwf[p * 64:(p + 1) * 64, j, :64] = blk
            wwf[p * 64:(p + 1) * 64, j, 64:] = blk

    bias = np.zeros((128, 2), np.float32)
    bias[:64, 0] = 8.0 * b2; bias[64:, 0] = 8.0 * b2
    bias[:64, 1] = bf; bias[64:, 1] = bf

    return {
        "wta": wta, "wtb": wtb, "wsum": wsum,
        "wsc": wsc, "we": we, "web": web, "wwf": wwf, "bias": bias,
        "idt8": np.eye(8, dtype=np.float32),
        "idt128": np.eye(128, dtype=np.float32),
    }


CONST_SHAPES = {
    "wta": (128, 2, 3, 128), "wtb": (128, 2, 3, 128),
    "wsum": (128, 128), "wsc": (128, 4, 8), "we": (8, 4, 128),
    "web": (8, 4, 128), "wwf": (128, 4, 128), "bias": (128, 2),
    "idt8": (8, 8), "idt128": (128, 128),
}


# ----------------------------------------------------------------------------
# kernel program
# ----------------------------------------------------------------------------

def build_nc():
    import concourse.bass as bass
    import concourse.tile as tile
    import concourse.mybir as mybir
    from concourse import bacc
    from contextlib import ExitStack

    f32 = mybir.dt.float32
    f32r = mybir.dt.float32r
    fp16 = mybir.dt.float16
    ACT = mybir.ActivationFunctionType
    ALU = mybir.AluOpType
    AX = mybir.AxisListType

    nc = bacc.Bacc("TRN2", target_bir_lowering=False, debug=False)

    bf16 = mybir.dt.bfloat16
    CDT = {"wta": f32r, "wtb": f32r, "wsum": f32r,
           "wsc": f32r, "we": f32r, "web": bf16, "wwf": f32r,
           "bias": f32, "idt8": f32, "idt128": f32}
    x = nc.dram_tensor("x", [B, 2, 128, T, 9, 130], f32r,
                       kind="ExternalInput").ap()
    cst = {n: nc.dram_tensor(n, list(s), CDT[n], kind="ExternalInput").ap()
           for n, s in CONST_SHAPES.items()}
    out = nc.dram_tensor("out", [B, 128, HS, W], f32, kind="ExternalOutput").ap()
    # DRAM scratch ring for the warm-unit attn transpose (DRAM->SBUF XBAR)
    atdr = nc.dram_tensor("atdr", [2, 128, HS, 16], bf16, kind="Internal").ap()

    with tile.TileContext(nc) as tc, ExitStack() as ctx:
        cp = ctx.enter_context(tc.tile_pool(name="const", bufs=1))
        wup_f = cp.tile([128, 512], f32, tag="wup_f")
        nc.gpsimd.memset(wup_f[:], 1.0)
        consts = {}

        def load_consts(names):
            for n in names:
                s = CONST_SHAPES[n]
                t = cp.tile(list(s), CDT[n], tag=n, name=f"c_{n}")
                nc.sync.dma_start(t[:], cst[n][:])
                consts[n] = t

        xx_pool = ctx.enter_context(tc.tile_pool(name="xx", bufs=5))
        ref_pool = ctx.enter_context(tc.tile_pool(name="ref", bufs=4))
        emb_pool = ctx.enter_context(tc.tile_pool(name="emb", bufs=2))
        sml_pool = ctx.enter_context(tc.tile_pool(name="sml", bufs=2))
        prod_pool = ctx.enter_context(tc.tile_pool(name="prodp", bufs=4))
        out_pool = ctx.enter_context(tc.tile_pool(name="outb", bufs=2))
        # PSUM: ps1 = 2 tiles x 2 banks (conv cps, double-buffered frames);
        # pss = 1 tile x 2 banks (emb sums / scps, unit-cycled);
        # ps3 = 2 tiles x 1 bank (psT / abc rotation).
        ps1 = ctx.enter_context(tc.tile_pool(name="ps1", bufs=2, space="PSUM"))
        pss = ctx.enter_context(tc.tile_pool(name="pss", bufs=1, space="PSUM"))
        ps3 = ctx.enter_context(tc.tile_pool(name="ps3", bufs=2, space="PSUM"))

        state = {}

        def r(ap):
            return ap.bitcast(f32r)

        def load_xx(u):
            b, s = divmod(u, 2)
            xxh = []
            for hf in range(2):
                xt_ = xx_pool.tile([128, 4, 9, 130], f32r, tag="xx",
                                   name=f"xx{u}_{hf}")
                nc.sync.dma_start(xt_[:], x[b, s, :, 4 * hf:4 * (hf + 1)])
                xxh.append(xt_)
            return xxh

        def emit_a(u, inject, pre=None, post=None):
            """Conv/scores phase for unit u; inject[t] () emitted after
            frame t's convs (PE-stream interleaving for unit u-1)."""
            b, s = divmod(u, 2)
            xxh = load_xx(u) if pre is None else pre

            def xx(t):
                return xxh[t // 4][:, t % 4]

            sums = pss.tile([128, NCH, 512], f32, tag="pss", name=f"sum{u}")
            refs = []
            embs = []
            for t in range(T):
                j, sw = divmod(t, 2)
                if sw == 0:
                    refj = ref_pool.tile([128, NCH, 512], f32, tag="embref",
                                         name=f"ref{u}_{j}")
                    embj = emb_pool.tile([128, NCH, 512], f32r, tag="emb",
                                         name=f"emb{u}_{j}")
                    refs.append(refj)
                    embs.append(embj)
                refj, embj = refs[j], embs[j]
                cps = ps1.tile([128, NCH, 512], f32, tag="cps",
                               name=f"cps{u}_{t}")
                for dj in range(3):
                    for ch in range(NCH):
                        nc.tensor.matmul(
                            cps[:, ch], consts['wta'][:, sw, dj, :],
                            xx(t)[:, 4 * ch:4 * ch + 4, dj:dj + 128],
                            start=(dj == 0), stop=False)
                for dj in range(3):
                    nc.tensor.matmul(
                        cps[:, 0], consts['wtb'][0:64, sw, dj, :],
                        xx(t)[0:64, 2:6, dj:dj + 128],
                        start=False, stop=(dj == 2))
                    nc.tensor.matmul(
                        cps[:, 1], consts['wtb'][64:128, sw, dj, :],
                        xx(t)[64:128, 5:9, dj:dj + 128],
                        start=False, stop=(dj == 2))
                if sw == 0:
                    nc.scalar.activation(refj[0:64], cps[0:64], ACT.Copy)
                    nc.scalar.activation(embj[64:128], cps[64:128], ACT.Copy)
                else:
                    nc.scalar.activation(embj[0:64], cps[0:64], ACT.Copy)
                    nc.scalar.activation(refj[64:128], cps[64:128], ACT.Copy)
                if sw == 1:
                    for ch in range(NCH):
                        nc.tensor.matmul(sums[:, ch], consts['wsum'][:],
                                         embj[:, ch, :],
                                         start=(j == 0), stop=(j == 3))
                fn = inject.get(t)
                if fn:
                    fn()

            if post:
                post()
            esum = sml_pool.tile([128, NCH, 512], f32, tag="esum",
                                 name=f"esum{u}")
            nc.scalar.activation(esum[:], sums[:], ACT.Identity,
                                 bias=consts['bias'][:, 0:1])
            scps = pss.tile([8, NCH, 512], f32, tag="pss", name=f"scps{u}")
            prods = []
            for j in range(4):
                prodj = prod_pool.tile([128, NCH, 512], f32r, tag="prod",
                                       name=f"prod{u}_{j}")
                nc.vector.tensor_mul(prodj[:].opt(), refs[j][:].opt(),
                                     esum[:].opt())
                prods.append(prodj)
            for j in range(4):
                for ch in range(NCH):
                    nc.tensor.matmul(scps[:, ch], consts['wsc'][:, j, :],
                                     prods[j][:, ch, :],
                                     start=(j == 0), stop=(j == 3))
            state[u] = {"xxh": xxh, "scps": scps}

        _fill_n = [0]

        def filler(pool, n=1):
            """HAM-warming matmuls on scratch PSUM banks of `pool`."""
            for _ in range(n):
                i = _fill_n[0]
                _fill_n[0] += 1
                fps = pool.tile([128, NCH, 512], f32,
                                tag="cps" if pool is ps1 else "pss",
                                name=f"fill{i}")
                nc.tensor.matmul(fps[:, 0], r(wup_f[:, 0:128]), r(wup_f[:]),
                                 start=True, stop=True)

        def b_pieces(u, tail=False):
            """B-phase for unit u as injectable pieces keyed by slot."""
            b, s = divmod(u, 2)
            st = state[u]

            def xx(t):
                return st["xxh"][t // 4][:, t % 4]

            def p_trans():
                scs = sml_pool.tile([8, NCH, 512], f32, tag="scs",
                                    name=f"scs{u}")
                nc.scalar.activation(scs[:], st["scps"][:], ACT.Copy)
                if tail:
                    filler(ps1, 3)
                psT = ps3.tile([128, HS, 8], f32, tag="ps3", name=f"psT{u}")
                for h in range(HS):
                    nc.tensor.transpose(
                        psT[:, h, :],
                        scs[:].opt()[:, 128 * h:128 * (h + 1)], consts['idt8'][:])
                st["psT"] = psT

            def p_soft():
                from concourse.bass import broadcast_tensor_aps
                psT = st["psT"]
                nm = sml_pool.tile([128, HS, 1], f32, tag="nm",
                                   name=f"nm{u}")
                nc.vector.tensor_reduce(nm[:, :, 0], psT[:], axis=AX.X,
                                        op=ALU.max)
                sm = sml_pool.tile([128, HS, 8], f32, tag="sm",
                                   name=f"sm{u}")
                a0, a1 = broadcast_tensor_aps(psT[:], nm[:])
                nc.vector.tensor_tensor(sm[:], a0, a1, op=ALU.subtract)
                et = sml_pool.tile([128, HS, 8], f32, tag="et",
                                   name=f"et{u}")
                nc.scalar.activation(et[:].opt(), sm[:].opt(), ACT.Exp)
                den = sml_pool.tile([128, HS, 1], f32, tag="den",
                                    name=f"den{u}")
                nc.vector.tensor_reduce(den[:, :, 0], et[:], axis=AX.X,
                                        op=ALU.add)
                rec = sml_pool.tile([128, HS, 1], f32, tag="rec",
                                    name=f"rec{u}")
                nc.vector.reciprocal(rec[:], den[:])
                if False:
                    attnT = sml_pool.tile([128, HS, 16], bf16, tag="attnT",
                                          name=f"attnT{u}")
                    # cols 8:16 transpose to attn partitions 8:16 (never
                    # read); zero them so the XBAR DMA doesn't move
                    # uninitialized SBUF
                    nc.gpsimd.memset(attnT[:, :, 8:16], 0.0)
                    b0, b1_ = broadcast_tensor_aps(et[:], rec[:])
                    nc.vector.tensor_tensor(attnT[:, :, 0:8], b0, b1_,
                                            op=ALU.mult)
                else:
                    attnT = sml_pool.tile([128, HS, 8], f32, tag="attnT",
                                          name=f"attnT{u}")
                    b0, b1_ = broadcast_tensor_aps(et[:], rec[:])
                    nc.vector.tensor_tensor(attnT[:], b0, b1_, op=ALU.mult)
                st["attnT"] = attnT

            def p_btrans():
                if False:
                    # bounce through DRAM; the XBAR transposing load is the
                    # HW-validated direction (SBUF->SBUF XBAR is not)
                    nc.sync.dma_start(atdr[u % 2], st["attnT"][:])
                    attn = sml_pool.tile([16, HS, 128], bf16, tag="attn",
                                         name=f"attn{u}")
                    nc.sync.dma_start_transpose(attn[:], atdr[u % 2])
                    st["attn"] = attn
                    return
                attnT = st["attnT"]
                attn_ps = [ps3.tile([8, 512], f32, tag="ps3",
                                    name=f"attnps{u}_{ch}")
                           for ch in range(NCH)]
                for h in range(HS):
                    nc.tensor.transpose(
                        attn_ps[h // 4][:, 128 * (h % 4):128 * (h % 4 + 1)],
                        attnT[:, h, :], consts['idt128'][:])
                attn = sml_pool.tile([8, HS, 128], f32r, tag="attn",
                                     name=f"attn{u}")
                for ch in range(NCH):
                    nc.scalar.activation(attn[0:8, 4 * ch:4 * (ch + 1), :],
                                         attn_ps[ch][:], ACT.Copy)
                st["attn"] = attn

            def t_abc(j):
                """abc matmuls for round j (and the ops alloc on round 0)."""
                attn = st["attn"]
                if j == 0:
                    # conv stream is over; ps1 banks are free for the
                    # 1x1-conv accumulator so wwf can chase the wt muls
                    st["ops"] = ps1.tile([128, NCH, 512], f32, tag="cps",
                                         name=f"ops{u}")
                abcs = st.setdefault("abcs", {})
                abcs[j] = []
                for ch in range(NCH):
                    # cycle 3 PSUM slots (2x ps3 + the dead scps slot in pss)
                    # so round j+1's abc never waits on round j's wt reads
                    pool = pss if (2 * j + ch) % 3 == 2 else ps3
                    abc = pool.tile([128, 4, 128], f32,
                                    tag="cps" if pool is ps1 else
                                    ("pss" if pool is pss else "ps3"),
                                    name=f"abc{u}_{j}_{ch}")
                    nc.tensor.matmul(abc[:].opt(), consts['we'][:, j, :],
                                     attn[0:8, 4 * ch:4 * (ch + 1), :].opt(),
                                     start=True, stop=True)
                    abcs[j].append(abc)

            def t_wt(j):
                wtsj = st.setdefault("wtsj", {})
                wtsj[j] = []
                for ch in range(NCH):
                    abc = st["abcs"][j][ch]
                    wt = sml_pool.tile([128, 4, 128], f32r, tag="wt",
                                       bufs=5, name=f"wt{u}_{j}_{ch}")
                    nc.vector.tensor_mul(
                        wt[0:64],
                        xx(2 * j)[0:64, 1 + 4 * ch:5 + 4 * ch, 1:129]
                        .bitcast(f32), abc[0:64])
                    nc.vector.tensor_mul(
                        wt[64:128],
                        xx(2 * j + 1)[64:128, 4 * ch:4 + 4 * ch, 1:129]
                        .bitcast(f32), abc[64:128])
                    wtsj[j].append(wt)

            def t_wwf(j):
                for ch in range(NCH):
                    nc.tensor.matmul(st["ops"][:, ch], consts['wwf'][:, j, :],
                                     st["wtsj"][j][ch][:].opt(),
                                     start=(j == 0), stop=(j == 3))

            def p_wj(j):
                def fn():
                    attn = st["attn"]
                    wts = st.setdefault("wts", [])
                    for ch in range(NCH):
                        abc = ps3.tile([128, 4, 128], f32, tag="ps3",
                                       name=f"abc{u}_{j}_{ch}")
                        nc.tensor.matmul(abc[:].opt(), consts['we'][:, j, :],
                                         attn[0:8, 4 * ch:4 * (ch + 1), :].opt(),
                                         start=True, stop=True)
                        wt = sml_pool.tile([128, 4, 128], f32r, tag="wt",
                                           bufs=5, name=f"wt{u}_{j}_{ch}")
                        nc.vector.tensor_mul(
                            wt[0:64],
                            xx(2 * j)[0:64, 1 + 4 * ch:5 + 4 * ch, 1:129]
                            .bitcast(f32), abc[0:64])
                        nc.vector.tensor_mul(
                            wt[64:128],
                            xx(2 * j + 1)[64:128, 4 * ch:4 + 4 * ch, 1:129]
                            .bitcast(f32), abc[64:128])
                        wts.append(wt)
                return fn

            def p_wf():
                wts = st["wts"]
                st["ops"] = ps1.tile([128, NCH, 512], f32, tag="cps",
                                     name=f"ops{u}")
                for j in range(4):
                    for ch in range(NCH):
                        nc.tensor.matmul(st["ops"][:, ch],
                                         consts['wwf'][:, j, :],
                                         wts[2 * j + ch][:].opt(),
                                         start=(j == 0), stop=(j == 3))
                finish()

            def finish():
                ops = st["ops"]
                outbuf = state.get(("ob", b))
                if outbuf is None:
                    outbuf = out_pool.tile([128, HS, W], f32, tag="outbuf",
                                           name=f"outbuf{b}")
                    state[("ob", b)] = outbuf
                yb = sml_pool.tile([128, NCH, 512], f32, tag="yb",
                                   name=f"yb{u}")
                chunks = [(0, NCH)] if not tail else [(0, 1), (1, 2)]
                for c0, c1 in chunks:
                    nc.scalar.activation(
                        yb[64 * s:64 * (s + 1), c0:c1],
                        ops[64 * s:64 * (s + 1), c0:c1],
                        ACT.Identity,
                        bias=consts['bias'][64 * s:64 * (s + 1), 1:2])
                    nc.vector.scalar_tensor_tensor(
                        outbuf[64 * s:64 * (s + 1),
                               4 * c0:4 * c1].opt(),
                        yb[64 * s:64 * (s + 1), c0:c1].opt(), 0.1,
                        yb[64 * s:64 * (s + 1), c0:c1].opt(),
                        op0=ALU.mult, op1=ALU.max)
                    nc.sync.dma_start(
                        out[b, 64 * s:64 * (s + 1), 4 * c0:4 * c1],
                        outbuf[64 * s:64 * (s + 1), 4 * c0:4 * c1])
                del state[u]
                if s == 1:
                    del state[("ob", b)]

            if tail:
                return {"trans": p_trans, "soft": p_soft, "btrans": p_btrans,
                        "abc": t_abc, "wt": t_wt, "wwf": t_wwf,
                        "finish": finish}
            return {0: p_trans, 1: p_soft, 2: p_btrans,
                    3: p_wj(0), 4: p_wj(1), 5: p_wj(2), 6: p_wj(3),
                    7: p_wf}

        # --- startup: weights first, then frame 0 of unit 0, warmup MMs ---
        load_consts(["wta"])
        b0_, s0_ = 0, 0
        xx0 = xx_pool.tile([128, 4, 9, 130], f32r, tag="xx", name="xx0_0")
        nc.sync.dma_start(xx0[:, 0], x[b0_, s0_, :, 0])
        load_consts(["wtb"])
        for f in range(1, 4):
            nc.sync.dma_start(xx0[:, f], x[b0_, s0_, :, f])
        xx1 = xx_pool.tile([128, 4, 9, 130], f32r, tag="xx", name="xx0_1")
        for f in range(4):
            nc.sync.dma_start(xx1[:, f], x[b0_, s0_, :, 4 + f])
        load_consts([n for n in CONST_SHAPES if n not in ("wta", "wtb")])

        wup = ps1.tile([128, NCH, 512], f32, tag="cps", name="wup")
        for i in range(16):
            nc.tensor.matmul(wup[:, 0], r(wup_f[:, 0:128]), r(wup_f[:]),
                             start=True, stop=True)

        emit_a(0, {}, [xx0, xx1])
        for u in range(1, 7):
            emit_a(u, b_pieces(u - 1))
        # fillers ride the PE through unit 7's esum/prods window so HAM
        # stays at K=8/8 into the drain
        emit_a(7, b_pieces(6), post=lambda: filler(ps1, 6))
        # drain last unit: abc -> wt -> wwf software-pipelined, fillers in
        # every PE hole (ps1 until `ops` is allocated, pss afterwards)
        tp = b_pieces(7, tail=True)
        tp["trans"]()
        filler(ps1, 6)
        tp["soft"]()
        filler(ps1, 6)
        tp["btrans"]()
        filler(ps1, 5)
        tp["abc"](0)
        tp["wt"](0)
        tp["abc"](1)
        tp["wt"](1)
        tp["wwf"](0)
        tp["abc"](2)
        tp["wt"](2)
        tp["wwf"](1)
        tp["abc"](3)
        tp["wt"](3)
        tp["wwf"](2)
        tp["wwf"](3)
        tp["finish"]()

    nc.compile()
    return nc


# ----------------------------------------------------------------------------
# entry point
# ----------------------------------------------------------------------------

def kernel(aligned_feat, w1, b1, w2, b2, wf, bf):
    from concourse import bass_utils

    if "nc" not in _CACHE:
        _CACHE["nc"] = build_nc()
    nc = _CACHE["nc"]

    A = _stage_inputs(aligned_feat)
    consts = _make_consts(w1, b1, w2, b2, wf, bf)
    in_maps = [{"x": A[k], **consts} for k in range(NCORES)]

    res = bass_utils.run_bass_kernel_spmd(nc, in_maps, core_ids=list(range(NCORES)))
    outs = [res.results[k]["out"] for k in range(NCORES)]  # [B,128,HS,W]

    full = np.empty((B, C, H, W), np.float32)
    for k in range(NCORES):
        o = outs[k]
        for s in range(2):
            full[:, :, RPC * k + HS * s: RPC * k + HS * (s + 1), :] = \
                o[:, 64 * s:64 * (s + 1), :, :]
    return full
